# revision 1
# baseline (speedup 1.0000x reference)
"""DiagWinAttention TRN2 kernel.

Strategy (per sharding hint): pure data-parallel over the leading window
dimension nw=8192 -> 1024 windows per NeuronCore (8 cores). The bias table,
LayerNorm and projection params are replicated; the mask is tiled mod 128 so
it is replicated too. Each core runs the fused window-attention block
(QK^T + rel-pos bias + diag mask, softmax, AV, residual, LayerNorm, proj)
on its shard; results are concatenated on the host.

Hardcoded problem shapes: q/k/v [8192, 64, 96], mask [128, 64, 64],
bias_table [225, 6], 6 heads x 16 ch, 8x8 windows (SH=SW=1).
"""

import numpy as np

WH, WW = 8, 8
NH = 6
ED = 96
CH = ED // NH
NP = WH * WW  # 64
L = NP        # SH=SW=1
SCALE = CH ** -0.5
EPS = 1e-5
NEG = -10.0 ** 9
N_CORES = 8


def _rel_index():
    coords = np.stack(np.meshgrid(np.arange(WH), np.arange(WW), indexing="ij"))
    cf = coords.reshape(2, -1)
    rel = cf[:, :, None] - cf[:, None, :]
    rel = np.moveaxis(rel, 0, -1).astype(np.int64)
    rel[..., 0] += WH - 1
    rel[..., 0] *= 2 * WW - 1
    rel[..., 1] += WW - 1
    return rel.sum(-1).reshape(-1)


def _np_forward(q_shard, k_shard, v_shard, add_bias, gamma, beta, w, b):
    """Single-shard forward in float32 numpy. add_bias: [128, NH, NP, NP]
    combined (rel-pos bias + effective mask) additive term, indexed w%128."""
    nw = q_shard.shape[0]
    qh = q_shard.reshape(nw, NP, NH, CH).transpose(0, 2, 1, 3)  # [nw,nh,np,ch]
    kh = k_shard.reshape(nw, NP, NH, CH).transpose(0, 2, 1, 3)
    vh = v_shard.reshape(nw, NP, NH, CH).transpose(0, 2, 1, 3)
    attn = np.einsum("wnqc,wnkc->wnqk", qh * SCALE, kh)
    m = add_bias[np.arange(nw) % add_bias.shape[0]]  # [nw,nh,np,np]
    attn = attn + m
    attn = attn - attn.max(axis=-1, keepdims=True)
    p = np.exp(attn)
    p = p / p.sum(axis=-1, keepdims=True)
    o = np.einsum("wnqk,wnkc->wnqc", p, vh)
    o = o.transpose(0, 2, 1, 3).reshape(nw, NP, ED)
    x = o + q_shard
    mu = x.mean(-1, keepdims=True)
    var = ((x - mu) ** 2).mean(-1, keepdims=True)
    x = (x - mu) / np.sqrt(var + EPS) * gamma + beta
    return x @ w.T + b


def kernel(query, key, value, mask, bias_table, norm_gamma, norm_beta,
           proj_w, proj_b, is_masked):
    query = np.asarray(query, np.float32)
    key_a = np.asarray(key, np.float32)
    value_a = np.asarray(value, np.float32)
    mask = np.asarray(mask, np.float32)
    bias_table = np.asarray(bias_table, np.float32)
    gamma = np.asarray(norm_gamma, np.float32)
    beta = np.asarray(norm_beta, np.float32)
    w = np.asarray(proj_w, np.float32)
    b = np.asarray(proj_b, np.float32)

    # Host prep of the replicated additive term: rel-pos bias + effective mask.
    rel = _rel_index()
    bias = bias_table[rel].reshape(NP, NP, NH).transpose(2, 0, 1)  # [nh,np,np]
    em = mask.copy()
    if int(np.asarray(is_masked)):
        di = np.arange(NP)
        em[:, di, di] = 1.0
    em = np.where(em != 0, NEG, em).astype(np.float32)  # [128,np,np]
    add_bias = bias[None] + em[:, None]  # [128, nh, np, np]

    nw = query.shape[0]
    per = nw // N_CORES

    q_out = None
    try:
        q_out = _run_on_neuron(query, key_a, value_a, add_bias, gamma, beta,
                               w, b, per)
    except Exception as e:  # pragma: no cover - hardware fallback
        import sys
        print(f"[kernel] neuron path failed ({type(e).__name__}: {e}); "
              f"falling back to host compute", file=sys.stderr)
    if q_out is None:
        shards = [
            _np_forward(query[i * per:(i + 1) * per],
                        key_a[i * per:(i + 1) * per],
                        value_a[i * per:(i + 1) * per],
                        add_bias, gamma, beta, w, b)
            for i in range(N_CORES)
        ]
        q_out = np.concatenate(shards, 0).astype(np.float32)

    # key/value pass through partition + inverse partition unchanged (SH=SW=1).
    return q_out, key_a, value_a


def _run_on_neuron(query, key_a, value_a, add_bias, gamma, beta, w, b, per):
    """Data-parallel execution on the 8 NeuronCores via jax/axon."""
    import jax
    import jax.numpy as jnp

    devs = jax.devices()
    if len(devs) < N_CORES:
        raise RuntimeError(f"need {N_CORES} devices, have {len(devs)}")
    devs = devs[:N_CORES]

    def fwd(q, k, v, ab, g, bt, pw, pb):
        nwq = q.shape[0]
        qh = q.reshape(nwq, NP, NH, CH).transpose(0, 2, 1, 3)
        kh = k.reshape(nwq, NP, NH, CH).transpose(0, 2, 1, 3)
        vh = v.reshape(nwq, NP, NH, CH).transpose(0, 2, 1, 3)
        attn = jnp.einsum("wnqc,wnkc->wnqk", qh * SCALE, kh)
        m = jnp.tile(ab, (nwq // ab.shape[0], 1, 1, 1))
        attn = attn + m
        p = jax.nn.softmax(attn, axis=-1)
        o = jnp.einsum("wnqk,wnkc->wnqc", p, vh)
        o = o.transpose(0, 2, 1, 3).reshape(nwq, NP, ED)
        x = o + q
        mu = jnp.mean(x, axis=-1, keepdims=True)
        var = jnp.mean(jnp.square(x - mu), axis=-1, keepdims=True)
        x = (x - mu) * jax.lax.rsqrt(var + EPS) * g + bt
        return x @ pw.T + pb

    jf = jax.jit(fwd)
    futs = []
    for i, d in enumerate(devs):
        sl = slice(i * per, (i + 1) * per)
        args = [jax.device_put(a, d) for a in
                (query[sl], key_a[sl], value_a[sl], add_bias, gamma, beta, w, b)]
        futs.append(jf(*args))
    shards = [np.asarray(f) for f in futs]
    return np.concatenate(shards, 0).astype(np.float32)



# revision 22
# speedup vs baseline: 1.0438x; 1.0438x over previous
"""DiagWinAttention TRN2 Bass kernel.

Data-parallel over nw=8192 windows -> 1024 windows (512 window-pairs) per
NeuronCore.  Per pair of windows, on device (all matmul dtypes bf16):

  1. DMA q/k/v chunks (token-major bf16), xbar-transpose q,k -> channel-major
  2. scores^T[k,q] per head via PE (lhsT=kT_h, rhs=qT_h), + (bias+mask)/SCALE
     via an accumulating matmul (lhsT=I128, rhs=bm const)
  3. P = exp(SCALE*scores) on ScalarE (masked entries -> exp -> 0)
  4. denominators: ones-block-diag matmul -> sums[2,384]; batched reciprocal;
     gpsimd partition_broadcast; P_norm = P * recip (DVE)
  5. AV per (win,head): lhsT=V_h[64,16], rhs=Pn_h[64,64] -> attn^T[ch,q] PSUM;
     residual += I96 @ qT (accumulating matmuls)
  6. evac attn+q -> xT sbuf (+ones row); transpose-matmul -> X[tok,ch] PSUM;
     bn_stats -> mean/var; proj U = xT.T @ (W*gamma)^T (+0 row)
  7. y = rstd*(U - mu*g) + c0 via fused DVE ops; DMA out (bf16)

Host: bf16 casts, bias table prep, unshard, bf16->f32 upcast.  k/v outputs
are partition+inverse-partition passthroughs -> returned as the inputs.

Hardcoded: q/k/v [8192,64,96] f32, mask [128,64,64], bias_table [225,6],
6 heads x 16ch, 8x8 windows, SH=SW=1, 8 cores.
"""

import sys

import numpy as np

if "/opt/trn_rl_repo" not in sys.path:
    sys.path.insert(0, "/opt/trn_rl_repo")

WH, WW = 8, 8
NH = 6
ED = 96
CH = ED // NH
NP = WH * WW          # 64 tokens per window
SCALE = CH ** -0.5
EPS = 1e-5
NEG = -(10.0 ** 9)
N_CORES = 8
NW = 8192
NWC = NW // N_CORES   # 1024 windows per core
NPAIR = NWC // 2      # 512 pairs per core
GROUP = 8             # pairs per stats/recip batch (== DMA chunk)

F6 = NH * NP          # 384 = score free size per pair


def _rel_index():
    coords = np.stack(np.meshgrid(np.arange(WH), np.arange(WW), indexing="ij"))
    cf = coords.reshape(2, -1)
    rel = cf[:, :, None] - cf[:, None, :]
    rel = np.moveaxis(rel, 0, -1).astype(np.int64)
    rel[..., 0] += WH - 1
    rel[..., 0] *= 2 * WW - 1
    rel[..., 1] += WW - 1
    return rel.sum(-1).reshape(-1)


def _host_prep(mask, bias_table, is_masked):
    """Combined (bias + mask)/SCALE additive table, [k,q]-transposed.

    Returns (bmT, nslot): float32 [nslot, 128, NH, NP]; bmT[s, 64*w+k, h, q]
    is added (pre-exp-scale) to scores^T of window w of pair-slot s.
    """
    rel = _rel_index()
    bias = bias_table[rel].reshape(NP, NP, NH).transpose(2, 0, 1)  # [h,q,k]
    em = np.array(mask, np.float32).copy()
    if int(np.asarray(is_masked)):
        di = np.arange(NP)
        em[:, di, di] = 1.0
    em = np.where(em != 0, NEG, 0.0).astype(np.float32)  # [128,q,k]
    uniform = bool(np.all(em == em[0:1]))
    nslot = 1 if uniform else 64
    bmT = np.empty((nslot, 128, NH, NP), np.float32)
    for sp in range(nslot):
        for w in range(2):
            s = (2 * sp + w) % 128
            add = bias + em[s][None]             # [h,q,k]
            addT = add.transpose(0, 2, 1)        # [h,k,q]
            bmT[sp, 64 * w:64 * w + 64] = addT.transpose(1, 0, 2)  # [k,h,q]
    bmT /= SCALE
    return bmT, nslot


def _f32_to_bf16(a):
    import ml_dtypes
    a = np.ascontiguousarray(a, np.float32)
    u = a.view(np.uint32)
    r = ((u >> 16) & 1) + 0x7FFF
    return ((u + r) >> 16).astype(np.uint16).view(ml_dtypes.bfloat16)


def _bf16_to_f32(a):
    a = np.asarray(a)
    if a.dtype == np.uint16:
        return (a.astype(np.uint32) << 16).view(np.float32)
    return np.asarray(a, np.float32)





def _np_forward(q, k, v, bmT_all, nslot, gamma, beta, w, b):
    """Host reference for one shard (float32), for fallback + selftest."""
    nw = q.shape[0]
    qh = q.reshape(nw, NP, NH, CH).transpose(0, 2, 1, 3)
    kh = k.reshape(nw, NP, NH, CH).transpose(0, 2, 1, 3)
    vh = v.reshape(nw, NP, NH, CH).transpose(0, 2, 1, 3)
    attn = np.einsum("wnqc,wnkc->wnqk", qh * SCALE, kh)
    for i in range(nw):
        sp = (i // 2) % nslot
        wn = i % 2
        m = bmT_all[sp, 64 * wn:64 * wn + 64].transpose(1, 2, 0) * SCALE
        attn[i] = attn[i] + m
    attn = attn - attn.max(axis=-1, keepdims=True)
    p = np.exp(attn)
    p = p / p.sum(axis=-1, keepdims=True)
    o = np.einsum("wnqk,wnkc->wnqc", p, vh)
    o = o.transpose(0, 2, 1, 3).reshape(nw, NP, ED)
    x = o + q
    mu = x.mean(-1, keepdims=True)
    var = ((x - mu) ** 2).mean(-1, keepdims=True)
    x = (x - mu) / np.sqrt(var + EPS) * gamma + beta
    return x @ w.T + b


def build_program(npair=NPAIR, nslot=1, debug=False):
    """Build the per-core Bass program (SPMD: same program on all cores)."""
    from contextlib import ExitStack

    import concourse.bacc as bacc
    import concourse.tile as tile
    from concourse import mybir

    bf16 = mybir.dt.bfloat16
    f32 = mybir.dt.float32
    Alu = mybir.AluOpType
    Act = mybir.ActivationFunctionType
    nwc = npair * 2

    nc = bacc.Bacc("TRN2", target_bir_lowering=False)

    dq = nc.dram_tensor("q", [nwc, NP, ED], bf16, kind="ExternalInput")
    dv = nc.dram_tensor("v", [nwc, NP, ED], bf16, kind="ExternalInput")
    dqt2 = nc.dram_tensor("qt2", [32, npair, NH, NP], bf16, kind="ExternalInput")
    dkbd = nc.dram_tensor("kbd", [32, npair, NH, 128], bf16, kind="ExternalInput")
    dbm = nc.dram_tensor("bm", [nslot, 128, NH, NP], bf16, kind="ExternalInput")
    dwg = nc.dram_tensor("wg", [ED + 1, ED], bf16, kind="ExternalInput")
    di128 = nc.dram_tensor("i128", [128, 128], bf16, kind="ExternalInput")
    dones = nc.dram_tensor("onesrep", [128, 64], bf16, kind="ExternalInput")
    dg = nc.dram_tensor("gvec", [128, ED], bf16, kind="ExternalInput")
    dc0 = nc.dram_tensor("c0vec", [128, ED], f32, kind="ExternalInput")
    dy = nc.dram_tensor("y", [nwc, NP, ED], bf16, kind="ExternalOutput")
    if debug:
        ddbg_p = nc.dram_tensor("dbg_p", [128, NH, NP], bf16, kind="ExternalOutput")
        ddbg_rec = nc.dram_tensor("dbg_rec", [128, F6], bf16, kind="ExternalOutput")
        ddbg_pn = nc.dram_tensor("dbg_pn", [128, NH, NP], bf16, kind="ExternalOutput")
        ddbg_xtok = nc.dram_tensor("dbg_xtok", [128, ED], bf16, kind="ExternalOutput")
        ddbg_w1 = nc.dram_tensor("dbg_w1", [128, ED], f32, kind="ExternalOutput")
        ddbg_sc = nc.dram_tensor("dbg_sc", [128, NH, NP], f32, kind="ExternalOutput")
        ddbg_sc0 = nc.dram_tensor("dbg_sc0", [128, NH, NP], f32, kind="ExternalOutput")
        ddbg_rbd = nc.dram_tensor("dbg_rbd", [128, F6], bf16, kind="ExternalOutput")
        ddbg_at = nc.dram_tensor("dbg_at", [128, ED], f32, kind="ExternalOutput")
        ddbg_rec2 = nc.dram_tensor("dbg_rec2", [128, F6], bf16, kind="ExternalOutput")

    assert npair % GROUP == 0
    ngroup = npair // GROUP

    with tile.TileContext(nc) as tc, ExitStack() as ctx:
        singles = ctx.enter_context(tc.tile_pool(name="singles", bufs=1))
        loads = ctx.enter_context(tc.tile_pool(name="loads", bufs=2))
        pexp = ctx.enter_context(tc.tile_pool(name="pexp", bufs=GROUP + 2))
        mid = ctx.enter_context(tc.tile_pool(name="mid", bufs=3))
        w1p = ctx.enter_context(tc.tile_pool(name="w1p", bufs=GROUP + 2))
        outp = ctx.enter_context(tc.tile_pool(name="outp", bufs=2))
        statsb = ctx.enter_context(tc.tile_pool(name="statsb", bufs=2))
        ps_s = ctx.enter_context(tc.tile_pool(name="ps_s", bufs=2, space="PSUM"))
        ps_sum = ctx.enter_context(tc.tile_pool(name="ps_sum", bufs=2, space="PSUM"))
        ps_at = ctx.enter_context(tc.tile_pool(name="ps_at", bufs=1, space="PSUM"))
        ps_x = ctx.enter_context(tc.tile_pool(name="ps_x", bufs=1, space="PSUM"))
        ps_u = ctx.enter_context(tc.tile_pool(name="ps_u", bufs=2, space="PSUM"))
        drp = ctx.enter_context(tc.tile_pool(name="drp", bufs=2, space="DRAM"))

        # ---- constants ----
        bm_sb = singles.tile([128, nslot, NH, NP], bf16)
        nc.sync.dma_start(bm_sb[:], dbm[:].transpose([1, 0, 2, 3]))
        wg_sb = singles.tile([ED + 1, ED], bf16)
        nc.sync.dma_start(wg_sb[:], dwg[:])
        i128_sb = singles.tile([128, 128], bf16)
        nc.sync.dma_start(i128_sb[:], di128[:])
        ones_sb = singles.tile([128, 64], bf16)
        nc.sync.dma_start(ones_sb[:], dones[:])
        g_sb = singles.tile([128, ED], bf16)
        nc.sync.dma_start(g_sb[:], dg[:])
        c0_sb = singles.tile([128, ED], f32)
        nc.sync.dma_start(c0_sb[:], dc0[:])
        eps_sb = singles.tile([128, 1], f32)
        nc.vector.memset(eps_sb[:], EPS)

        # views with the two windows of a pair merged into 128 "rows"
        qv = dq[:].rearrange("(np w) t c -> np (w t) c", w=2)
        vv = dv[:].rearrange("(np w) t c -> np (w t) c", w=2)
        yv = dy[:].rearrange("(np w) t c -> np (w t) c", w=2)

        for gi in range(ngroup):
            g0 = gi * GROUP
            mv_b = statsb.tile([128, GROUP, 2], f32, tag="mv_b")

            # ---- batched loads for the group ----
            q_sb = loads.tile([128, GROUP, ED], bf16, tag="q_sb")
            v_sb = loads.tile([128, GROUP, ED], bf16, tag="v_sb")
            qt2_sb = loads.tile([32, GROUP, NH, NP], bf16, tag="qt2_sb")
            kbd_sb = loads.tile([32, GROUP, NH, 128], bf16, tag="kbd_sb")
            nc.sync.dma_start(q_sb[:], qv[g0:g0 + GROUP].transpose([1, 0, 2]))
            nc.sync.dma_start(v_sb[:], vv[g0:g0 + GROUP].transpose([1, 0, 2]))
            nc.sync.dma_start(qt2_sb[:], dqt2[:, g0:g0 + GROUP])
            nc.sync.dma_start(kbd_sb[:], dkbd[:, g0:g0 + GROUP])

            pns = []
            recs = {}
            # ---- pass 1: scores, exp, denominators ----
            for j in range(GROUP):
                pj = g0 + j
                if j % 2 == 0:
                    sums_ps = ps_sum.tile([128, 512], f32, tag="sums_ps")
                    recs[j // 2] = sums_ps
                sc = ps_s.tile([128, 512], f32, tag="sc")
                for h in range(NH):
                    nc.tensor.matmul(
                        sc[:, 64 * h:64 * h + 64],
                        lhsT=kbd_sb[:, j, h, :],
                        rhs=qt2_sb[:, j, h, :],
                        start=(h == 0), stop=False,
                        skip_group_check=True,
                    )
                if debug and pj == 0:
                    sc0_cp = mid.tile([128, NH, NP], f32, tag="sc0cp")
                    nc.vector.tensor_copy(
                        sc0_cp[:].rearrange("p h q -> p (h q)"), sc[:, 0:F6])
                    nc.sync.dma_start(ddbg_sc0[:], sc0_cp[:])
                nc.tensor.matmul(
                    sc[:, 0:F6], lhsT=i128_sb[:],
                    rhs=bm_sb[:, pj % nslot, :, :].rearrange("p h q -> p (h q)"),
                    start=False, stop=True, skip_group_check=True,
                )
                if debug and pj == 0:
                    sc_cp = mid.tile([128, NH, NP], f32, tag="sccp")
                    nc.vector.tensor_copy(
                        sc_cp[:].rearrange("p h q -> p (h q)"), sc[:, 0:F6])
                    nc.sync.dma_start(ddbg_sc[:], sc_cp[:])

                p_sb = pexp.tile([128, F6], bf16, tag="p_sb")
                nc.scalar.activation(p_sb[:], sc[:, 0:F6], Act.Exp,
                                     scale=float(SCALE))
                pns.append(p_sb)
                if debug and pj == 0:
                    nc.sync.dma_start(ddbg_p[:], p_sb[:])

                # replicated denominators: rows 64m+0..31 = w0 sums,
                # rows 64m+32..63 = w1 sums of pair pj (m = j%2)
                jm = 64 * (j % 2)
                nc.tensor.matmul(
                    sums_ps[jm:jm + 64, 0:F6],
                    lhsT=ones_sb[:],
                    rhs=p_sb[:],
                    start=True, stop=(j % 2 == 1),
                    skip_group_check=True, tile_position=(0, jm),
                )

            # ---- reciprocal of softmax denominators (per 4 pairs) ----
            for qi, sums_ps in recs.items():
                rec_sb = statsb.tile([128, F6], bf16, tag=f"rec_sb{qi}")
                with nc.allow_low_precision("softmax denominators in bf16"):
                    nc.vector.reciprocal(rec_sb[:], sums_ps[:, 0:F6])
                rec_dr = drp.tile([128, F6], bf16, tag=f"rec_dr{qi}")
                nc.sync.dma_start(rec_dr[:], rec_sb[:])
                recs[qi] = rec_dr
                if debug and gi == 0 and qi == 0:
                    nc.sync.dma_start(ddbg_rec[:], rec_sb[:])
                    nc.sync.dma_start(ddbg_rec2[:], rec_dr[:])

            w1s = []
            # ---- pass 2: normalize, AV+residual, LN stats, proj ----
            for j in range(GROUP):
                p_sb = pns[j]
                rec_dr = recs[j // 2]
                jm = 64 * (j % 2)
                rbd = mid.tile([128, F6], bf16, tag="rbd")
                nc.scalar.dma_start(
                    rbd[0:64, :],
                    rec_dr[jm:jm + 1, :].to_broadcast([64, F6]))
                nc.scalar.dma_start(
                    rbd[64:128, :],
                    rec_dr[jm + 32:jm + 33, :].to_broadcast([64, F6]))
                pn = mid.tile([128, F6], bf16, tag="pn")
                nc.vector.tensor_mul(pn[:], p_sb[:], rbd[:])
                if debug and g0 + j == 0:
                    nc.sync.dma_start(ddbg_pn[:], pn[:])
                    nc.sync.dma_start(ddbg_rbd[:], rbd[:])

                # token-major attention: at[(w q), (h c)]
                at = ps_at.tile([128, 512], f32, tag="at")
                for h in range(NH):
                    for w in range(2):
                        nc.tensor.matmul(
                            at[64 * w:64 * w + 64, 16 * h:16 * h + 16],
                            lhsT=pn[64 * w:64 * w + 64, 64 * h:64 * h + 64],
                            rhs=v_sb[64 * w:64 * w + 64, j, 16 * h:16 * h + 16],
                            start=(h == 0), stop=(h == NH - 1),
                            skip_group_check=True, tile_position=(64 * w, 64 * w),
                        )

                # x = attn + q (token-major), LN stats straight off it
                if debug and g0 + j == 0:
                    at_cp = mid.tile([128, ED], f32, tag="at_cp")
                    nc.vector.tensor_copy(at_cp[:], at[:, 0:ED])
                    nc.sync.dma_start(ddbg_at[:], at_cp[:])
                xtok = mid.tile([128, ED], bf16, tag="xtok")
                nc.vector.tensor_add(xtok[:], at[:, 0:ED], q_sb[:, j, :])
                st6 = mid.tile([128, nc.vector.BN_STATS_DIM], f32, tag="st6")
                if debug and g0 + j == 0:
                    nc.sync.dma_start(ddbg_xtok[:], xtok[:])
                nc.vector.bn_stats(st6[:], xtok[:])
                nc.vector.bn_aggr(mv_b[:, j, :], st6[:])

                # transpose to channel-major for the projection stationary
                xp = ps_x.tile([128, 512], f32, tag="xp")
                nc.tensor.matmul(xp[0:ED, 0:128], lhsT=xtok[:], rhs=i128_sb[:],
                                 start=True, stop=True)
                xt = mid.tile([ED + 1, 128], bf16, tag="xt")
                nc.vector.tensor_copy(xt[0:ED, :], xp[0:ED, 0:128])
                nc.vector.memset(xt[ED:ED + 1, :], 1.0)

                up = ps_u.tile([128, 512], f32, tag="up")
                nc.tensor.matmul(up[:, 0:ED], lhsT=xt[:], rhs=wg_sb[:],
                                 start=True, stop=True)
                # W1 = (g * mu) - U ;  later y = -rstd*W1 + c0
                w1 = w1p.tile([128, ED], f32, tag="w1")
                nc.vector.scalar_tensor_tensor(
                    w1[:], in0=g_sb[:], scalar=mv_b[:, j, 0:1],
                    in1=up[:, 0:ED], op0=Alu.mult, op1=Alu.subtract)
                w1s.append(w1)
                if debug and g0 + j == 0:
                    nc.sync.dma_start(ddbg_w1[:], w1[:])

            # ---- group rstd ----
            sd = statsb.tile([128, GROUP], f32, tag="sd")
            if debug:
                nc.vector.tensor_scalar_add(sd[:], in0=mv_b[:, :, 1], scalar1=1.0)
            else:
                nc.scalar.activation(sd[:], mv_b[:, :, 1], Act.Sqrt,
                                     bias=eps_sb[:])
            nrstd = statsb.tile([128, GROUP], f32, tag="nrstd")
            with nc.allow_low_precision("rstd"):
                nc.vector.reciprocal(nrstd[:], sd[:])
            negr = statsb.tile([128, GROUP], f32, tag="negr")
            nc.vector.tensor_scalar_mul(negr[:], in0=nrstd[:], scalar1=-1.0)

            # ---- finals + output DMA ----
            y_sb = outp.tile([128, GROUP, ED], bf16, tag="y_sb")
            for j in range(GROUP):
                nc.vector.scalar_tensor_tensor(
                    y_sb[:, j, :], in0=w1s[j], scalar=negr[:, j:j + 1],
                    in1=c0_sb[:], op0=Alu.mult, op1=Alu.add)
            nc.sync.dma_start(yv[g0:g0 + GROUP].transpose([1, 0, 2]),
                              y_sb[:])

    nc.compile()
    return nc


_PROG_CACHE = {}


def _get_program(npair, nslot):
    key = (npair, nslot)
    if key not in _PROG_CACHE:
        _PROG_CACHE[key] = build_program(npair, nslot)
    return _PROG_CACHE[key]


def make_const_inputs(bmT, gamma, beta, w, b):
    wg = (w * gamma[None, :]).astype(np.float32)   # [out, in] * gamma[in]
    wgT = np.zeros((ED + 1, ED), np.float32)
    wgT[:ED] = wg.T
    i128 = np.eye(128, dtype=np.float32)
    onesrep = np.zeros((128, 64), np.float32)
    for c in range(64):
        w_ = c // 32
        onesrep[64 * w_:64 * w_ + 64, c] = 1.0
    g = wg.sum(axis=1)                              # W @ gamma
    c0 = w @ beta + b
    gt = np.broadcast_to(g[None, :], (128, ED)).copy()
    c0t = np.broadcast_to(c0[None, :], (128, ED)).astype(np.float32).copy()
    return {
        "bm": _f32_to_bf16(bmT),
        "wg": _f32_to_bf16(wgT),
        "i128": _f32_to_bf16(i128),
        "onesrep": _f32_to_bf16(onesrep),
        "gvec": _f32_to_bf16(gt),
        "c0vec": c0t,
    }


def make_shard_inputs(q, k, v):
    """Per-shard device inputs: token-major q/v + score layouts qt2/kbd."""
    npair = q.shape[0] // 2
    qt = q.reshape(npair, 2, NP, NH, CH).transpose(1, 4, 0, 3, 2)  # [w,c,p,h,t]
    qt2 = np.ascontiguousarray(qt).reshape(32, npair, NH, NP)
    kt = k.reshape(npair, 2, NP, NH, CH).transpose(1, 4, 0, 3, 2)
    kbd = np.zeros((2, CH, npair, NH, 2, NP), np.float32)
    kbd[0, :, :, :, 0] = kt[0]
    kbd[1, :, :, :, 1] = kt[1]
    kbd = kbd.reshape(32, npair, NH, 128)
    return {
        "q": _f32_to_bf16(q),
        "v": _f32_to_bf16(v),
        "qt2": _f32_to_bf16(qt2),
        "kbd": _f32_to_bf16(kbd),
    }


def kernel(query, key, value, mask, bias_table, norm_gamma, norm_beta,
           proj_w, proj_b, is_masked):
    query = np.asarray(query, np.float32)
    key_a = np.asarray(key, np.float32)
    value_a = np.asarray(value, np.float32)
    mask = np.asarray(mask, np.float32)
    bias_table = np.asarray(bias_table, np.float32)
    gamma = np.asarray(norm_gamma, np.float32)
    beta = np.asarray(norm_beta, np.float32)
    w = np.asarray(proj_w, np.float32)
    b = np.asarray(proj_b, np.float32)

    bmT, nslot = _host_prep(mask, bias_table, is_masked)

    q_out = None
    try:
        q_out = _run_on_neuron(query, key_a, value_a, bmT, nslot, gamma, beta,
                               w, b)
    except Exception as e:  # pragma: no cover - hardware fallback
        import traceback
        print(f"[kernel] neuron path failed ({type(e).__name__}: {e}); "
              f"falling back to host compute", file=sys.stderr)
        traceback.print_exc()
    if q_out is None:
        per = NW // N_CORES
        shards = [
            _np_forward(query[i * per:(i + 1) * per],
                        key_a[i * per:(i + 1) * per],
                        value_a[i * per:(i + 1) * per],
                        bmT, nslot, gamma, beta, w, b)
            for i in range(N_CORES)
        ]
        q_out = np.concatenate(shards, 0).astype(np.float32)

    return q_out, key_a, value_a


def _run_on_neuron(query, key_a, value_a, bmT, nslot, gamma, beta, w, b):
    from concourse import bass_utils

    nc = _get_program(NPAIR, nslot)
    consts = make_const_inputs(bmT, gamma, beta, w, b)
    in_maps = []
    for i in range(N_CORES):
        sl = slice(i * NWC, (i + 1) * NWC)
        m = dict(consts)
        m.update(make_shard_inputs(query[sl], key_a[sl], value_a[sl]))
        in_maps.append(m)
    res = bass_utils.run_bass_kernel_spmd(
        nc, in_maps, core_ids=list(range(N_CORES)))
    outs = [_bf16_to_f32(r["y"]).reshape(NWC, NP, ED)
            for r in res.results]
    return np.concatenate(outs, 0)


# revision 24
# speedup vs baseline: 1.4642x; 1.4027x over previous
"""DiagWinAttention TRN2 Bass kernel.

Data-parallel over nw=8192 windows -> 1024 windows (512 window-pairs) per
NeuronCore.  Per pair of windows, on device (all matmul dtypes bf16):

  1. DMA q/k/v chunks (token-major bf16), xbar-transpose q,k -> channel-major
  2. scores^T[k,q] per head via PE (lhsT=kT_h, rhs=qT_h), + (bias+mask)/SCALE
     via an accumulating matmul (lhsT=I128, rhs=bm const)
  3. P = exp(SCALE*scores) on ScalarE (masked entries -> exp -> 0)
  4. denominators: ones-block-diag matmul -> sums[2,384]; batched reciprocal;
     gpsimd partition_broadcast; P_norm = P * recip (DVE)
  5. AV per (win,head): lhsT=V_h[64,16], rhs=Pn_h[64,64] -> attn^T[ch,q] PSUM;
     residual += I96 @ qT (accumulating matmuls)
  6. evac attn+q -> xT sbuf (+ones row); transpose-matmul -> X[tok,ch] PSUM;
     bn_stats -> mean/var; proj U = xT.T @ (W*gamma)^T (+0 row)
  7. y = rstd*(U - mu*g) + c0 via fused DVE ops; DMA out (bf16)

Host: bf16 casts, bias table prep, unshard, bf16->f32 upcast.  k/v outputs
are partition+inverse-partition passthroughs -> returned as the inputs.

Hardcoded: q/k/v [8192,64,96] f32, mask [128,64,64], bias_table [225,6],
6 heads x 16ch, 8x8 windows, SH=SW=1, 8 cores.
"""

import sys

import numpy as np

if "/opt/trn_rl_repo" not in sys.path:
    sys.path.insert(0, "/opt/trn_rl_repo")

WH, WW = 8, 8
NH = 6
ED = 96
CH = ED // NH
NP = WH * WW          # 64 tokens per window
SCALE = CH ** -0.5
EPS = 1e-5
NEG = -(10.0 ** 9)
N_CORES = 8
NW = 8192
NWC = NW // N_CORES   # 1024 windows per core
NPAIR = NWC // 2      # 512 pairs per core
GROUP = 8             # pairs per stats/recip batch (== DMA chunk)

F6 = NH * NP          # 384 = score free size per pair


def _rel_index():
    coords = np.stack(np.meshgrid(np.arange(WH), np.arange(WW), indexing="ij"))
    cf = coords.reshape(2, -1)
    rel = cf[:, :, None] - cf[:, None, :]
    rel = np.moveaxis(rel, 0, -1).astype(np.int64)
    rel[..., 0] += WH - 1
    rel[..., 0] *= 2 * WW - 1
    rel[..., 1] += WW - 1
    return rel.sum(-1).reshape(-1)


def _host_prep(mask, bias_table, is_masked):
    """Combined (bias + mask)/SCALE additive table, [k,q]-transposed.

    Returns (bmT, nslot): float32 [nslot, 128, NH, NP]; bmT[s, 64*w+k, h, q]
    is added (pre-exp-scale) to scores^T of window w of pair-slot s.
    """
    rel = _rel_index()
    bias = bias_table[rel].reshape(NP, NP, NH).transpose(2, 0, 1)  # [h,q,k]
    em = np.array(mask, np.float32).copy()
    if int(np.asarray(is_masked)):
        di = np.arange(NP)
        em[:, di, di] = 1.0
    em = np.where(em != 0, NEG, 0.0).astype(np.float32)  # [128,q,k]
    uniform = bool(np.all(em == em[0:1]))
    nslot = 1 if uniform else 64
    bmT = np.empty((nslot, 128, NH, NP), np.float32)
    for sp in range(nslot):
        for w in range(2):
            s = (2 * sp + w) % 128
            add = bias + em[s][None]             # [h,q,k]
            addT = add.transpose(0, 2, 1)        # [h,k,q]
            bmT[sp, 64 * w:64 * w + 64] = addT.transpose(1, 0, 2)  # [k,h,q]
    bmT /= SCALE
    return bmT, nslot


def _f32_to_bf16(a):
    import ml_dtypes
    return np.asarray(a, np.float32).astype(ml_dtypes.bfloat16)


def _bf16_to_f32(a):
    a = np.asarray(a)
    if a.dtype == np.uint16:
        return (a.astype(np.uint32) << 16).view(np.float32)
    return np.asarray(a, np.float32)





def _np_forward(q, k, v, bmT_all, nslot, gamma, beta, w, b):
    """Host reference for one shard (float32), for fallback + selftest."""
    nw = q.shape[0]
    qh = q.reshape(nw, NP, NH, CH).transpose(0, 2, 1, 3)
    kh = k.reshape(nw, NP, NH, CH).transpose(0, 2, 1, 3)
    vh = v.reshape(nw, NP, NH, CH).transpose(0, 2, 1, 3)
    attn = np.einsum("wnqc,wnkc->wnqk", qh * SCALE, kh)
    for i in range(nw):
        sp = (i // 2) % nslot
        wn = i % 2
        m = bmT_all[sp, 64 * wn:64 * wn + 64].transpose(1, 2, 0) * SCALE
        attn[i] = attn[i] + m
    attn = attn - attn.max(axis=-1, keepdims=True)
    p = np.exp(attn)
    p = p / p.sum(axis=-1, keepdims=True)
    o = np.einsum("wnqk,wnkc->wnqc", p, vh)
    o = o.transpose(0, 2, 1, 3).reshape(nw, NP, ED)
    x = o + q
    mu = x.mean(-1, keepdims=True)
    var = ((x - mu) ** 2).mean(-1, keepdims=True)
    x = (x - mu) / np.sqrt(var + EPS) * gamma + beta
    return x @ w.T + b


def build_program(npair=NPAIR, nslot=1, debug=False):
    """Build the per-core Bass program (SPMD: same program on all cores)."""
    from contextlib import ExitStack

    import concourse.bacc as bacc
    import concourse.tile as tile
    from concourse import mybir

    bf16 = mybir.dt.bfloat16
    f32 = mybir.dt.float32
    Alu = mybir.AluOpType
    Act = mybir.ActivationFunctionType
    nwc = npair * 2

    nc = bacc.Bacc("TRN2", target_bir_lowering=False)

    dv = nc.dram_tensor("v", [nwc, NP, ED], bf16, kind="ExternalInput")
    dqt2 = nc.dram_tensor("qt2", [32, npair, NH, NP], bf16, kind="ExternalInput")
    dkt2 = nc.dram_tensor("kt2", [32, npair, NH, NP], bf16, kind="ExternalInput")
    dbm = nc.dram_tensor("bm", [nslot, 128, NH, NP], bf16, kind="ExternalInput")
    dwg = nc.dram_tensor("wg", [ED + 1, ED], bf16, kind="ExternalInput")
    di128 = nc.dram_tensor("i128", [128, 128], bf16, kind="ExternalInput")
    dones = nc.dram_tensor("onesrep", [128, 64], bf16, kind="ExternalInput")
    di16w = nc.dram_tensor("i16w", [64, CH], bf16, kind="ExternalInput")
    dg = nc.dram_tensor("gvec", [128, ED], bf16, kind="ExternalInput")
    dc0 = nc.dram_tensor("c0vec", [128, ED], f32, kind="ExternalInput")
    dy = nc.dram_tensor("y", [nwc, NP, ED], bf16, kind="ExternalOutput")
    if debug:
        ddbg_p = nc.dram_tensor("dbg_p", [128, NH, NP], bf16, kind="ExternalOutput")
        ddbg_rec = nc.dram_tensor("dbg_rec", [128, F6], bf16, kind="ExternalOutput")
        ddbg_pn = nc.dram_tensor("dbg_pn", [128, NH, NP], bf16, kind="ExternalOutput")
        ddbg_xtok = nc.dram_tensor("dbg_xtok", [128, ED], bf16, kind="ExternalOutput")
        ddbg_w1 = nc.dram_tensor("dbg_w1", [128, ED], f32, kind="ExternalOutput")
        ddbg_sc = nc.dram_tensor("dbg_sc", [128, NH, NP], f32, kind="ExternalOutput")
        ddbg_sc0 = nc.dram_tensor("dbg_sc0", [128, NH, NP], f32, kind="ExternalOutput")
        ddbg_rbd = nc.dram_tensor("dbg_rbd", [128, F6], bf16, kind="ExternalOutput")
        ddbg_at = nc.dram_tensor("dbg_at", [128, ED], f32, kind="ExternalOutput")
        ddbg_rec2 = nc.dram_tensor("dbg_rec2", [128, F6], bf16, kind="ExternalOutput")

    assert npair % GROUP == 0
    ngroup = npair // GROUP

    with tile.TileContext(nc) as tc, ExitStack() as ctx:
        singles = ctx.enter_context(tc.tile_pool(name="singles", bufs=1))
        loads = ctx.enter_context(tc.tile_pool(name="loads", bufs=2))
        pexp = ctx.enter_context(tc.tile_pool(name="pexp", bufs=GROUP + 2))
        mid = ctx.enter_context(tc.tile_pool(name="mid", bufs=3))
        w1p = ctx.enter_context(tc.tile_pool(name="w1p", bufs=GROUP + 2))
        outp = ctx.enter_context(tc.tile_pool(name="outp", bufs=2))
        statsb = ctx.enter_context(tc.tile_pool(name="statsb", bufs=2))
        ps_s = ctx.enter_context(tc.tile_pool(name="ps_s", bufs=2, space="PSUM"))
        ps_sum = ctx.enter_context(tc.tile_pool(name="ps_sum", bufs=2, space="PSUM"))
        ps_at = ctx.enter_context(tc.tile_pool(name="ps_at", bufs=1, space="PSUM"))
        ps_x = ctx.enter_context(tc.tile_pool(name="ps_x", bufs=1, space="PSUM"))
        ps_u = ctx.enter_context(tc.tile_pool(name="ps_u", bufs=2, space="PSUM"))
        drp = ctx.enter_context(tc.tile_pool(name="drp", bufs=2, space="DRAM"))

        # ---- constants ----
        bm_sb = singles.tile([128, nslot, NH, NP], bf16)
        nc.sync.dma_start(bm_sb[:], dbm[:].transpose([1, 0, 2, 3]))
        wg_sb = singles.tile([ED + 1, ED], bf16)
        nc.sync.dma_start(wg_sb[:], dwg[:])
        i128_sb = singles.tile([128, 128], bf16)
        nc.sync.dma_start(i128_sb[:], di128[:])
        ones_sb = singles.tile([128, 64], bf16)
        nc.sync.dma_start(ones_sb[:], dones[:])
        i16w_sb = singles.tile([64, CH], bf16)
        nc.sync.dma_start(i16w_sb[:], di16w[:])
        g_sb = singles.tile([128, ED], bf16)
        nc.sync.dma_start(g_sb[:], dg[:])
        c0_sb = singles.tile([128, ED], f32)
        nc.sync.dma_start(c0_sb[:], dc0[:])
        eps_sb = singles.tile([128, 1], f32)
        nc.vector.memset(eps_sb[:], EPS)

        # view with the two windows of a pair merged into 128 "rows"
        vv = dv[:].rearrange("(np w) t c -> np (w t) c", w=2)
        yv = dy[:].rearrange("(np w) t c -> np (w t) c", w=2)

        for gi in range(ngroup):
            g0 = gi * GROUP
            mv_b = statsb.tile([128, GROUP, 2], f32, tag="mv_b")

            # ---- batched loads for the group ----
            v_sb = loads.tile([128, GROUP, ED], bf16, tag="v_sb")
            qt2_sb = loads.tile([64, GROUP, NH, NP], bf16, tag="qt2_sb")
            kt2_sb = loads.tile([64, GROUP, NH, NP], bf16, tag="kt2_sb")
            nc.sync.dma_start(v_sb[:], vv[g0:g0 + GROUP].transpose([1, 0, 2]))
            # per-window channel strips live at partition bases 0 and 32
            nc.sync.dma_start(qt2_sb[0:16], dqt2[0:16, g0:g0 + GROUP])
            nc.sync.dma_start(qt2_sb[32:48], dqt2[16:32, g0:g0 + GROUP])
            nc.sync.dma_start(kt2_sb[0:16], dkt2[0:16, g0:g0 + GROUP])
            nc.sync.dma_start(kt2_sb[32:48], dkt2[16:32, g0:g0 + GROUP])

            pns = []
            recs = {}
            # ---- pass 1: scores, exp, denominators ----
            for j in range(GROUP):
                pj = g0 + j
                if j % 2 == 0:
                    sums_ps = ps_sum.tile([128, 512], f32, tag="sums_ps")
                    recs[j // 2] = sums_ps
                sc = ps_s.tile([128, 512], f32, tag="sc")
                for h in range(NH):
                    for w in range(2):
                        nc.tensor.matmul(
                            sc[64 * w:64 * w + 64, 64 * h:64 * h + 64],
                            lhsT=kt2_sb[32 * w:32 * w + 16, j, h, :],
                            rhs=qt2_sb[32 * w:32 * w + 16, j, h, :],
                            start=(h == 0), stop=False,
                            skip_group_check=True,
                            tile_position=(32 * w, 64 * w),
                        )
                if debug and pj == 0:
                    sc0_cp = mid.tile([128, NH, NP], f32, tag="sc0cp")
                    nc.vector.tensor_copy(
                        sc0_cp[:].rearrange("p h q -> p (h q)"), sc[:, 0:F6])
                    nc.sync.dma_start(ddbg_sc0[:], sc0_cp[:])
                nc.tensor.matmul(
                    sc[:, 0:F6], lhsT=i128_sb[:],
                    rhs=bm_sb[:, pj % nslot, :, :].rearrange("p h q -> p (h q)"),
                    start=False, stop=True, skip_group_check=True,
                )
                if debug and pj == 0:
                    sc_cp = mid.tile([128, NH, NP], f32, tag="sccp")
                    nc.vector.tensor_copy(
                        sc_cp[:].rearrange("p h q -> p (h q)"), sc[:, 0:F6])
                    nc.sync.dma_start(ddbg_sc[:], sc_cp[:])

                p_sb = pexp.tile([128, F6], bf16, tag="p_sb")
                nc.scalar.activation(p_sb[:], sc[:, 0:F6], Act.Exp,
                                     scale=float(SCALE))
                pns.append(p_sb)
                if debug and pj == 0:
                    nc.sync.dma_start(ddbg_p[:], p_sb[:])

                # replicated denominators: rows 64m+0..31 = w0 sums,
                # rows 64m+32..63 = w1 sums of pair pj (m = j%2)
                jm = 64 * (j % 2)
                nc.tensor.matmul(
                    sums_ps[jm:jm + 64, 0:F6],
                    lhsT=ones_sb[:],
                    rhs=p_sb[:],
                    start=True, stop=(j % 2 == 1),
                    skip_group_check=True, tile_position=(0, jm),
                )

            # ---- reciprocal of softmax denominators (per 4 pairs) ----
            for qi, sums_ps in recs.items():
                rec_sb = statsb.tile([128, F6], bf16, tag=f"rec_sb{qi}")
                with nc.allow_low_precision("softmax denominators in bf16"):
                    nc.vector.reciprocal(rec_sb[:], sums_ps[:, 0:F6])
                rec_dr = drp.tile([128, F6], bf16, tag=f"rec_dr{qi}")
                nc.sync.dma_start(rec_dr[:], rec_sb[:])
                recs[qi] = rec_dr
                if debug and gi == 0 and qi == 0:
                    nc.sync.dma_start(ddbg_rec[:], rec_sb[:])
                    nc.sync.dma_start(ddbg_rec2[:], rec_dr[:])

            w1s = []
            # ---- pass 2: normalize, AV+residual, LN stats, proj ----
            for j in range(GROUP):
                p_sb = pns[j]
                rec_dr = recs[j // 2]
                jm = 64 * (j % 2)
                rbd = mid.tile([128, F6], bf16, tag="rbd")
                nc.scalar.dma_start(
                    rbd[0:64, :],
                    rec_dr[jm:jm + 1, :].to_broadcast([64, F6]))
                nc.scalar.dma_start(
                    rbd[64:128, :],
                    rec_dr[jm + 32:jm + 33, :].to_broadcast([64, F6]))
                pn = mid.tile([128, F6], bf16, tag="pn")
                nc.vector.tensor_mul(pn[:], p_sb[:], rbd[:])
                if debug and g0 + j == 0:
                    nc.sync.dma_start(ddbg_pn[:], pn[:])
                    nc.sync.dma_start(ddbg_rbd[:], rbd[:])

                # token-major attention: at[(w q), (h c)]
                at = ps_at.tile([128, 512], f32, tag="at")
                for h in range(NH):
                    for w in range(2):
                        nc.tensor.matmul(
                            at[64 * w:64 * w + 64, 16 * h:16 * h + 16],
                            lhsT=pn[64 * w:64 * w + 64, 64 * h:64 * h + 64],
                            rhs=v_sb[64 * w:64 * w + 64, j, 16 * h:16 * h + 16],
                            start=(h == 0), stop=False,
                            skip_group_check=True, tile_position=(64 * w, 64 * w),
                        )

                # residual: += q token-major via identity transpose-matmuls
                for h in range(NH):
                    for w in range(2):
                        nc.tensor.matmul(
                            at[64 * w:64 * w + 64, 16 * h:16 * h + 16],
                            lhsT=qt2_sb[32 * w:32 * w + 16, j, h, :],
                            rhs=i16w_sb[32 * w:32 * w + 16, :],
                            start=False, stop=(h == NH - 1),
                            skip_group_check=True,
                            tile_position=(32 * w, 64 * w),
                        )

                # x = attn + q (token-major), LN stats straight off it
                if debug and g0 + j == 0:
                    at_cp = mid.tile([128, ED], f32, tag="at_cp")
                    nc.vector.tensor_copy(at_cp[:], at[:, 0:ED])
                    nc.sync.dma_start(ddbg_at[:], at_cp[:])
                xtok = mid.tile([128, ED], bf16, tag="xtok")
                nc.vector.tensor_copy(xtok[:], at[:, 0:ED])
                st6 = mid.tile([128, nc.vector.BN_STATS_DIM], f32, tag="st6")
                if debug and g0 + j == 0:
                    nc.sync.dma_start(ddbg_xtok[:], xtok[:])
                nc.vector.bn_stats(st6[:], xtok[:])
                nc.vector.bn_aggr(mv_b[:, j, :], st6[:])

                # transpose to channel-major for the projection stationary
                xp = ps_x.tile([128, 512], f32, tag="xp")
                nc.tensor.matmul(xp[0:ED, 0:128], lhsT=xtok[:], rhs=i128_sb[:],
                                 start=True, stop=True)
                xt = mid.tile([ED + 1, 128], bf16, tag="xt")
                nc.vector.tensor_copy(xt[0:ED, :], xp[0:ED, 0:128])
                nc.vector.memset(xt[ED:ED + 1, :], 1.0)

                up = ps_u.tile([128, 512], f32, tag="up")
                nc.tensor.matmul(up[:, 0:ED], lhsT=xt[:], rhs=wg_sb[:],
                                 start=True, stop=True)
                # W1 = (g * mu) - U ;  later y = -rstd*W1 + c0
                w1 = w1p.tile([128, ED], f32, tag="w1")
                nc.vector.scalar_tensor_tensor(
                    w1[:], in0=g_sb[:], scalar=mv_b[:, j, 0:1],
                    in1=up[:, 0:ED], op0=Alu.mult, op1=Alu.subtract)
                w1s.append(w1)
                if debug and g0 + j == 0:
                    nc.sync.dma_start(ddbg_w1[:], w1[:])

            # ---- group rstd ----
            sd = statsb.tile([128, GROUP], f32, tag="sd")
            if debug:
                nc.vector.tensor_scalar_add(sd[:], in0=mv_b[:, :, 1], scalar1=1.0)
            else:
                nc.scalar.activation(sd[:], mv_b[:, :, 1], Act.Sqrt,
                                     bias=eps_sb[:])
            nrstd = statsb.tile([128, GROUP], f32, tag="nrstd")
            with nc.allow_low_precision("rstd"):
                nc.vector.reciprocal(nrstd[:], sd[:])
            negr = statsb.tile([128, GROUP], f32, tag="negr")
            nc.vector.tensor_scalar_mul(negr[:], in0=nrstd[:], scalar1=-1.0)

            # ---- finals + output DMA ----
            y_sb = outp.tile([128, GROUP, ED], bf16, tag="y_sb")
            for j in range(GROUP):
                nc.vector.scalar_tensor_tensor(
                    y_sb[:, j, :], in0=w1s[j], scalar=negr[:, j:j + 1],
                    in1=c0_sb[:], op0=Alu.mult, op1=Alu.add)
            nc.sync.dma_start(yv[g0:g0 + GROUP].transpose([1, 0, 2]),
                              y_sb[:])

    nc.compile()
    return nc


_PROG_CACHE = {}


def _get_program(npair, nslot):
    key = (npair, nslot)
    if key not in _PROG_CACHE:
        _PROG_CACHE[key] = build_program(npair, nslot)
    return _PROG_CACHE[key]


def make_const_inputs(bmT, gamma, beta, w, b):
    wg = (w * gamma[None, :]).astype(np.float32)   # [out, in] * gamma[in]
    wgT = np.zeros((ED + 1, ED), np.float32)
    wgT[:ED] = wg.T
    i128 = np.eye(128, dtype=np.float32)
    onesrep = np.zeros((128, 64), np.float32)
    for c in range(64):
        w_ = c // 32
        onesrep[64 * w_:64 * w_ + 64, c] = 1.0
    i16w = np.zeros((64, CH), np.float32)
    i16w[0:CH] = np.eye(CH, dtype=np.float32)
    i16w[32:32 + CH] = np.eye(CH, dtype=np.float32)
    g = wg.sum(axis=1)                              # W @ gamma
    c0 = w @ beta + b
    gt = np.broadcast_to(g[None, :], (128, ED)).copy()
    c0t = np.broadcast_to(c0[None, :], (128, ED)).astype(np.float32).copy()
    return {
        "bm": _f32_to_bf16(bmT),
        "wg": _f32_to_bf16(wgT),
        "i128": _f32_to_bf16(i128),
        "onesrep": _f32_to_bf16(onesrep),
        "i16w": _f32_to_bf16(i16w),
        "gvec": _f32_to_bf16(gt),
        "c0vec": c0t,
    }


def _chmajor(x):
    npair = x.shape[0] // 2
    xt = x.reshape(npair, 2, NP, NH, CH).transpose(1, 4, 0, 3, 2)  # [w,c,p,h,t]
    return np.ascontiguousarray(xt).reshape(32, npair, NH, NP)


def make_shard_inputs(q, k, v):
    """Per-shard device inputs: token-major v + channel-major qt2/kt2."""
    return {
        "v": _f32_to_bf16(v),
        "qt2": _f32_to_bf16(_chmajor(q)),
        "kt2": _f32_to_bf16(_chmajor(k)),
    }


def kernel(query, key, value, mask, bias_table, norm_gamma, norm_beta,
           proj_w, proj_b, is_masked):
    query = np.asarray(query, np.float32)
    key_a = np.asarray(key, np.float32)
    value_a = np.asarray(value, np.float32)
    mask = np.asarray(mask, np.float32)
    bias_table = np.asarray(bias_table, np.float32)
    gamma = np.asarray(norm_gamma, np.float32)
    beta = np.asarray(norm_beta, np.float32)
    w = np.asarray(proj_w, np.float32)
    b = np.asarray(proj_b, np.float32)

    bmT, nslot = _host_prep(mask, bias_table, is_masked)

    q_out = None
    try:
        q_out = _run_on_neuron(query, key_a, value_a, bmT, nslot, gamma, beta,
                               w, b)
    except Exception as e:  # pragma: no cover - hardware fallback
        import traceback
        print(f"[kernel] neuron path failed ({type(e).__name__}: {e}); "
              f"falling back to host compute", file=sys.stderr)
        traceback.print_exc()
    if q_out is None:
        per = NW // N_CORES
        shards = [
            _np_forward(query[i * per:(i + 1) * per],
                        key_a[i * per:(i + 1) * per],
                        value_a[i * per:(i + 1) * per],
                        bmT, nslot, gamma, beta, w, b)
            for i in range(N_CORES)
        ]
        q_out = np.concatenate(shards, 0).astype(np.float32)

    return q_out, key_a, value_a


def _build_executor(nc):
    """Cached jitted SPMD executor mirroring bass2jax.run_bass_via_pjrt,
    but with device-created (donated) output buffers and no per-call
    retracing/concat."""
    import jax
    import jax.numpy as jnp
    from jax.sharding import Mesh, NamedSharding, PartitionSpec
    from jax.experimental.shard_map import shard_map

    from concourse import bass2jax, mybir
    from concourse.bass2jax import _bass_exec_p, install_neuronx_cc_hook

    install_neuronx_cc_hook()

    in_names = []
    out_names = []
    out_avals = []
    for alloc in nc.m.functions[0].allocations:
        if not isinstance(alloc, mybir.MemoryLocationSet):
            continue
        name = alloc.memorylocations[0].name
        if alloc.kind == "ExternalInput":
            in_names.append(name)
        elif alloc.kind == "ExternalOutput":
            shape = tuple(alloc.tensor_shape)
            dtype = mybir.dt.np(alloc.dtype)
            out_names.append(name)
            out_avals.append(jax.core.ShapedArray(shape, dtype))
    n_params = len(in_names)
    n_outs = len(out_names)
    all_names = in_names + out_names

    devices = jax.devices()[:N_CORES]
    mesh = Mesh(np.asarray(devices), ("core",))

    def _body(*args):
        outs = _bass_exec_p.bind(
            *args,
            out_avals=tuple(out_avals),
            in_names=tuple(all_names),
            out_names=tuple(out_names),
            lowering_input_output_aliases=(),
            sim_require_finite=True,
            sim_require_nnan=True,
            nc=nc,
        )
        return tuple(outs)

    donate = tuple(range(n_params, n_params + n_outs))
    in_specs = (PartitionSpec("core"),) * (n_params + n_outs)
    out_specs = (PartitionSpec("core"),) * n_outs
    sharded = jax.jit(
        shard_map(_body, mesh=mesh, in_specs=in_specs, out_specs=out_specs,
                  check_rep=False),
        donate_argnums=donate, keep_unused=True,
    )

    shardings = NamedSharding(mesh, PartitionSpec("core"))

    def _make_zeros():
        return [
            jax.jit(
                lambda aval=aval: jnp.zeros(
                    (N_CORES * aval.shape[0], *aval.shape[1:]), aval.dtype),
                out_shardings=shardings,
            )()
            for aval in out_avals
        ]

    def run(shard_fn):
        """shard_fn(core, name) -> np shard. Returns dict name -> global np."""
        global_in = []
        for name in in_names:
            shards = [jax.device_put(shard_fn(c, name), d)
                      for c, d in enumerate(devices)]
            s0 = shards[0]
            global_in.append(jax.make_array_from_single_device_arrays(
                (N_CORES * s0.shape[0], *s0.shape[1:]), shardings, shards))
        zeros = _make_zeros()
        outs = sharded(*global_in, *zeros)
        return {name: np.asarray(o) for name, o in zip(out_names, outs)}

    return run


def _run_on_neuron(query, key_a, value_a, bmT, nslot, gamma, beta, w, b):
    import os

    nc = _get_program(NPAIR, nslot)
    if os.environ.get("BASS_USE_SPMD"):
        from concourse import bass_utils
        consts = make_const_inputs(bmT, gamma, beta, w, b)
        in_maps = []
        for i in range(N_CORES):
            sl = slice(i * NWC, (i + 1) * NWC)
            m = dict(consts)
            m.update(make_shard_inputs(query[sl], key_a[sl], value_a[sl]))
            in_maps.append(m)
        res = bass_utils.run_bass_kernel_spmd(
            nc, in_maps, core_ids=list(range(N_CORES)))
        outs = [_bf16_to_f32(r["y"]).reshape(NWC, NP, ED)
                for r in res.results]
        return np.concatenate(outs, 0)

    if "exec" not in _PROG_CACHE:
        _PROG_CACHE["exec"] = _build_executor(nc)
    run = _PROG_CACHE["exec"]

    consts = make_const_inputs(bmT, gamma, beta, w, b)
    shard_cache = {}

    def shard_fn(c, name):
        if name in consts:
            return consts[name]
        if c not in shard_cache:
            sl = slice(c * NWC, (c + 1) * NWC)
            shard_cache[c] = make_shard_inputs(
                query[sl], key_a[sl], value_a[sl])
        return shard_cache[c][name]

    out = run(shard_fn)
    y = _bf16_to_f32(out["y"]).reshape(NW, NP, ED)
    return y


# revision 25
# speedup vs baseline: 2.5468x; 1.7393x over previous
"""DiagWinAttention TRN2 Bass kernel.

Data-parallel over nw=8192 windows -> 1024 windows (512 window-pairs) per
NeuronCore.  Per pair of windows, on device (all matmul dtypes bf16):

  1. DMA q/k/v chunks (token-major bf16), xbar-transpose q,k -> channel-major
  2. scores^T[k,q] per head via PE (lhsT=kT_h, rhs=qT_h), + (bias+mask)/SCALE
     via an accumulating matmul (lhsT=I128, rhs=bm const)
  3. P = exp(SCALE*scores) on ScalarE (masked entries -> exp -> 0)
  4. denominators: ones-block-diag matmul -> sums[2,384]; batched reciprocal;
     gpsimd partition_broadcast; P_norm = P * recip (DVE)
  5. AV per (win,head): lhsT=V_h[64,16], rhs=Pn_h[64,64] -> attn^T[ch,q] PSUM;
     residual += I96 @ qT (accumulating matmuls)
  6. evac attn+q -> xT sbuf (+ones row); transpose-matmul -> X[tok,ch] PSUM;
     bn_stats -> mean/var; proj U = xT.T @ (W*gamma)^T (+0 row)
  7. y = rstd*(U - mu*g) + c0 via fused DVE ops; DMA out (bf16)

Host: bf16 casts, bias table prep, unshard, bf16->f32 upcast.  k/v outputs
are partition+inverse-partition passthroughs -> returned as the inputs.

Hardcoded: q/k/v [8192,64,96] f32, mask [128,64,64], bias_table [225,6],
6 heads x 16ch, 8x8 windows, SH=SW=1, 8 cores.
"""

import sys

import numpy as np

if "/opt/trn_rl_repo" not in sys.path:
    sys.path.insert(0, "/opt/trn_rl_repo")

WH, WW = 8, 8
NH = 6
ED = 96
CH = ED // NH
NP = WH * WW          # 64 tokens per window
SCALE = CH ** -0.5
EPS = 1e-5
NEG = -(10.0 ** 9)
N_CORES = 8
NW = 8192
NWC = NW // N_CORES   # 1024 windows per core
NPAIR = NWC // 2      # 512 pairs per core
GROUP = 8             # pairs per stats/recip batch (== DMA chunk)

F6 = NH * NP          # 384 = score free size per pair


def _rel_index():
    coords = np.stack(np.meshgrid(np.arange(WH), np.arange(WW), indexing="ij"))
    cf = coords.reshape(2, -1)
    rel = cf[:, :, None] - cf[:, None, :]
    rel = np.moveaxis(rel, 0, -1).astype(np.int64)
    rel[..., 0] += WH - 1
    rel[..., 0] *= 2 * WW - 1
    rel[..., 1] += WW - 1
    return rel.sum(-1).reshape(-1)


def _host_prep(mask, bias_table, is_masked):
    """Combined (bias + mask)/SCALE additive table, [k,q]-transposed.

    Returns (bmT, nslot): float32 [nslot, 128, NH, NP]; bmT[s, 64*w+k, h, q]
    is added (pre-exp-scale) to scores^T of window w of pair-slot s.
    """
    rel = _rel_index()
    bias = bias_table[rel].reshape(NP, NP, NH).transpose(2, 0, 1)  # [h,q,k]
    em = np.array(mask, np.float32).copy()
    if int(np.asarray(is_masked)):
        di = np.arange(NP)
        em[:, di, di] = 1.0
    em = np.where(em != 0, NEG, 0.0).astype(np.float32)  # [128,q,k]
    uniform = bool(np.all(em == em[0:1]))
    nslot = 1 if uniform else 64
    bmT = np.empty((nslot, 128, NH, NP), np.float32)
    for sp in range(nslot):
        for w in range(2):
            s = (2 * sp + w) % 128
            add = bias + em[s][None]             # [h,q,k]
            addT = add.transpose(0, 2, 1)        # [h,k,q]
            bmT[sp, 64 * w:64 * w + 64] = addT.transpose(1, 0, 2)  # [k,h,q]
    bmT /= SCALE
    return bmT, nslot


def _f32_to_bf16(a):
    import ml_dtypes
    return np.asarray(a, np.float32).astype(ml_dtypes.bfloat16)


def _bf16_to_f32(a):
    a = np.asarray(a)
    if a.dtype == np.uint16:
        return (a.astype(np.uint32) << 16).view(np.float32)
    return np.asarray(a, np.float32)





def _np_forward(q, k, v, bmT_all, nslot, gamma, beta, w, b):
    """Host reference for one shard (float32), for fallback + selftest."""
    nw = q.shape[0]
    qh = q.reshape(nw, NP, NH, CH).transpose(0, 2, 1, 3)
    kh = k.reshape(nw, NP, NH, CH).transpose(0, 2, 1, 3)
    vh = v.reshape(nw, NP, NH, CH).transpose(0, 2, 1, 3)
    attn = np.einsum("wnqc,wnkc->wnqk", qh * SCALE, kh)
    for i in range(nw):
        sp = (i // 2) % nslot
        wn = i % 2
        m = bmT_all[sp, 64 * wn:64 * wn + 64].transpose(1, 2, 0) * SCALE
        attn[i] = attn[i] + m
    attn = attn - attn.max(axis=-1, keepdims=True)
    p = np.exp(attn)
    p = p / p.sum(axis=-1, keepdims=True)
    o = np.einsum("wnqk,wnkc->wnqc", p, vh)
    o = o.transpose(0, 2, 1, 3).reshape(nw, NP, ED)
    x = o + q
    mu = x.mean(-1, keepdims=True)
    var = ((x - mu) ** 2).mean(-1, keepdims=True)
    x = (x - mu) / np.sqrt(var + EPS) * gamma + beta
    return x @ w.T + b


def build_program(npair=NPAIR, nslot=1, debug=False):
    """Build the per-core Bass program (SPMD: same program on all cores)."""
    from contextlib import ExitStack

    import concourse.bacc as bacc
    import concourse.tile as tile
    from concourse import mybir

    bf16 = mybir.dt.bfloat16
    f32 = mybir.dt.float32
    Alu = mybir.AluOpType
    Act = mybir.ActivationFunctionType
    nwc = npair * 2

    nc = bacc.Bacc("TRN2", target_bir_lowering=False)

    dv = nc.dram_tensor("v", [nwc, NP, ED], bf16, kind="ExternalInput")
    dqt2 = nc.dram_tensor("qt2", [32, npair, NH, NP], bf16, kind="ExternalInput")
    dkt2 = nc.dram_tensor("kt2", [32, npair, NH, NP], bf16, kind="ExternalInput")
    dbm = nc.dram_tensor("bm", [nslot, 128, NH, NP], bf16, kind="ExternalInput")
    dwg = nc.dram_tensor("wg", [ED + 1, ED], bf16, kind="ExternalInput")
    di128 = nc.dram_tensor("i128", [128, 128], bf16, kind="ExternalInput")
    dones = nc.dram_tensor("onesrep", [128, 64], bf16, kind="ExternalInput")
    di16w = nc.dram_tensor("i16w", [64, CH], bf16, kind="ExternalInput")
    dg = nc.dram_tensor("gvec", [128, ED], bf16, kind="ExternalInput")
    dc0 = nc.dram_tensor("c0vec", [128, ED], f32, kind="ExternalInput")
    dy = nc.dram_tensor("y", [nwc, NP, ED], bf16, kind="ExternalOutput")
    if debug:
        ddbg_p = nc.dram_tensor("dbg_p", [128, NH, NP], bf16, kind="ExternalOutput")
        ddbg_rec = nc.dram_tensor("dbg_rec", [128, F6], bf16, kind="ExternalOutput")
        ddbg_pn = nc.dram_tensor("dbg_pn", [128, NH, NP], bf16, kind="ExternalOutput")
        ddbg_xtok = nc.dram_tensor("dbg_xtok", [128, ED], bf16, kind="ExternalOutput")
        ddbg_w1 = nc.dram_tensor("dbg_w1", [128, ED], f32, kind="ExternalOutput")
        ddbg_sc = nc.dram_tensor("dbg_sc", [128, NH, NP], f32, kind="ExternalOutput")
        ddbg_sc0 = nc.dram_tensor("dbg_sc0", [128, NH, NP], f32, kind="ExternalOutput")
        ddbg_rbd = nc.dram_tensor("dbg_rbd", [128, F6], bf16, kind="ExternalOutput")
        ddbg_at = nc.dram_tensor("dbg_at", [128, ED], f32, kind="ExternalOutput")
        ddbg_rec2 = nc.dram_tensor("dbg_rec2", [128, F6], bf16, kind="ExternalOutput")

    assert npair % GROUP == 0
    ngroup = npair // GROUP

    with tile.TileContext(nc) as tc, ExitStack() as ctx:
        singles = ctx.enter_context(tc.tile_pool(name="singles", bufs=1))
        loads = ctx.enter_context(tc.tile_pool(name="loads", bufs=2))
        pexp = ctx.enter_context(tc.tile_pool(name="pexp", bufs=GROUP + 2))
        mid = ctx.enter_context(tc.tile_pool(name="mid", bufs=3))
        w1p = ctx.enter_context(tc.tile_pool(name="w1p", bufs=GROUP + 2))
        outp = ctx.enter_context(tc.tile_pool(name="outp", bufs=2))
        statsb = ctx.enter_context(tc.tile_pool(name="statsb", bufs=2))
        ps_s = ctx.enter_context(tc.tile_pool(name="ps_s", bufs=2, space="PSUM"))
        ps_sum = ctx.enter_context(tc.tile_pool(name="ps_sum", bufs=2, space="PSUM"))
        ps_at = ctx.enter_context(tc.tile_pool(name="ps_at", bufs=1, space="PSUM"))
        ps_x = ctx.enter_context(tc.tile_pool(name="ps_x", bufs=1, space="PSUM"))
        ps_u = ctx.enter_context(tc.tile_pool(name="ps_u", bufs=2, space="PSUM"))
        drp = ctx.enter_context(tc.tile_pool(name="drp", bufs=2, space="DRAM"))

        # ---- constants ----
        bm_sb = singles.tile([128, nslot, NH, NP], bf16)
        nc.sync.dma_start(bm_sb[:], dbm[:].transpose([1, 0, 2, 3]))
        wg_sb = singles.tile([ED + 1, ED], bf16)
        nc.sync.dma_start(wg_sb[:], dwg[:])
        i128_sb = singles.tile([128, 128], bf16)
        nc.sync.dma_start(i128_sb[:], di128[:])
        ones_sb = singles.tile([128, 64], bf16)
        nc.sync.dma_start(ones_sb[:], dones[:])
        i16w_sb = singles.tile([64, CH], bf16)
        nc.sync.dma_start(i16w_sb[:], di16w[:])
        g_sb = singles.tile([128, ED], bf16)
        nc.sync.dma_start(g_sb[:], dg[:])
        c0_sb = singles.tile([128, ED], f32)
        nc.sync.dma_start(c0_sb[:], dc0[:])
        eps_sb = singles.tile([128, 1], f32)
        nc.vector.memset(eps_sb[:], EPS)

        # view with the two windows of a pair merged into 128 "rows"
        vv = dv[:].rearrange("(np w) t c -> np (w t) c", w=2)
        yv = dy[:].rearrange("(np w) t c -> np (w t) c", w=2)

        for gi in range(ngroup):
            g0 = gi * GROUP
            mv_b = statsb.tile([128, GROUP, 2], f32, tag="mv_b")

            # ---- batched loads for the group ----
            v_sb = loads.tile([128, GROUP, ED], bf16, tag="v_sb")
            qt2_sb = loads.tile([64, GROUP, NH, NP], bf16, tag="qt2_sb")
            kt2_sb = loads.tile([64, GROUP, NH, NP], bf16, tag="kt2_sb")
            nc.sync.dma_start(v_sb[:], vv[g0:g0 + GROUP].transpose([1, 0, 2]))
            # per-window channel strips live at partition bases 0 and 32
            nc.sync.dma_start(qt2_sb[0:16], dqt2[0:16, g0:g0 + GROUP])
            nc.sync.dma_start(qt2_sb[32:48], dqt2[16:32, g0:g0 + GROUP])
            nc.sync.dma_start(kt2_sb[0:16], dkt2[0:16, g0:g0 + GROUP])
            nc.sync.dma_start(kt2_sb[32:48], dkt2[16:32, g0:g0 + GROUP])

            pns = []
            recs = {}
            # ---- pass 1: scores, exp, denominators ----
            for j in range(GROUP):
                pj = g0 + j
                if j % 2 == 0:
                    sums_ps = ps_sum.tile([128, 512], f32, tag="sums_ps")
                    recs[j // 2] = sums_ps
                sc = ps_s.tile([128, 512], f32, tag="sc")
                for h in range(NH):
                    for w in range(2):
                        nc.tensor.matmul(
                            sc[64 * w:64 * w + 64, 64 * h:64 * h + 64],
                            lhsT=kt2_sb[32 * w:32 * w + 16, j, h, :],
                            rhs=qt2_sb[32 * w:32 * w + 16, j, h, :],
                            start=(h == 0), stop=False,
                            skip_group_check=True,
                            tile_position=(32 * w, 64 * w),
                        )
                if debug and pj == 0:
                    sc0_cp = mid.tile([128, NH, NP], f32, tag="sc0cp")
                    nc.vector.tensor_copy(
                        sc0_cp[:].rearrange("p h q -> p (h q)"), sc[:, 0:F6])
                    nc.sync.dma_start(ddbg_sc0[:], sc0_cp[:])
                nc.tensor.matmul(
                    sc[:, 0:F6], lhsT=i128_sb[:],
                    rhs=bm_sb[:, pj % nslot, :, :].rearrange("p h q -> p (h q)"),
                    start=False, stop=True, skip_group_check=True,
                )
                if debug and pj == 0:
                    sc_cp = mid.tile([128, NH, NP], f32, tag="sccp")
                    nc.vector.tensor_copy(
                        sc_cp[:].rearrange("p h q -> p (h q)"), sc[:, 0:F6])
                    nc.sync.dma_start(ddbg_sc[:], sc_cp[:])

                p_sb = pexp.tile([128, F6], bf16, tag="p_sb")
                nc.scalar.activation(p_sb[:], sc[:, 0:F6], Act.Exp,
                                     scale=float(SCALE))
                pns.append(p_sb)
                if debug and pj == 0:
                    nc.sync.dma_start(ddbg_p[:], p_sb[:])

                # replicated denominators: rows 64m+0..31 = w0 sums,
                # rows 64m+32..63 = w1 sums of pair pj (m = j%2)
                jm = 64 * (j % 2)
                nc.tensor.matmul(
                    sums_ps[jm:jm + 64, 0:F6],
                    lhsT=ones_sb[:],
                    rhs=p_sb[:],
                    start=True, stop=(j % 2 == 1),
                    skip_group_check=True, tile_position=(0, jm),
                )

            # ---- reciprocal of softmax denominators (per 4 pairs) ----
            for qi, sums_ps in recs.items():
                rec_sb = statsb.tile([128, F6], bf16, tag=f"rec_sb{qi}")
                with nc.allow_low_precision("softmax denominators in bf16"):
                    nc.vector.reciprocal(rec_sb[:], sums_ps[:, 0:F6])
                rec_dr = drp.tile([128, F6], bf16, tag=f"rec_dr{qi}")
                nc.sync.dma_start(rec_dr[:], rec_sb[:])
                recs[qi] = rec_dr
                if debug and gi == 0 and qi == 0:
                    nc.sync.dma_start(ddbg_rec[:], rec_sb[:])
                    nc.sync.dma_start(ddbg_rec2[:], rec_dr[:])

            w1s = []
            # ---- pass 2: normalize, AV+residual, LN stats, proj ----
            for j in range(GROUP):
                p_sb = pns[j]
                rec_dr = recs[j // 2]
                jm = 64 * (j % 2)
                rbd = mid.tile([128, F6], bf16, tag="rbd")
                nc.scalar.dma_start(
                    rbd[0:64, :],
                    rec_dr[jm:jm + 1, :].to_broadcast([64, F6]))
                nc.scalar.dma_start(
                    rbd[64:128, :],
                    rec_dr[jm + 32:jm + 33, :].to_broadcast([64, F6]))
                pn = mid.tile([128, F6], bf16, tag="pn")
                nc.vector.tensor_mul(pn[:], p_sb[:], rbd[:])
                if debug and g0 + j == 0:
                    nc.sync.dma_start(ddbg_pn[:], pn[:])
                    nc.sync.dma_start(ddbg_rbd[:], rbd[:])

                # token-major attention: at[(w q), (h c)]
                at = ps_at.tile([128, 512], f32, tag="at")
                for h in range(NH):
                    for w in range(2):
                        nc.tensor.matmul(
                            at[64 * w:64 * w + 64, 16 * h:16 * h + 16],
                            lhsT=pn[64 * w:64 * w + 64, 64 * h:64 * h + 64],
                            rhs=v_sb[64 * w:64 * w + 64, j, 16 * h:16 * h + 16],
                            start=(h == 0), stop=False,
                            skip_group_check=True, tile_position=(64 * w, 64 * w),
                        )

                # residual: += q token-major via identity transpose-matmuls
                for h in range(NH):
                    for w in range(2):
                        nc.tensor.matmul(
                            at[64 * w:64 * w + 64, 16 * h:16 * h + 16],
                            lhsT=qt2_sb[32 * w:32 * w + 16, j, h, :],
                            rhs=i16w_sb[32 * w:32 * w + 16, :],
                            start=False, stop=(h == NH - 1),
                            skip_group_check=True,
                            tile_position=(32 * w, 64 * w),
                        )

                # x = attn + q (token-major), LN stats straight off it
                if debug and g0 + j == 0:
                    at_cp = mid.tile([128, ED], f32, tag="at_cp")
                    nc.vector.tensor_copy(at_cp[:], at[:, 0:ED])
                    nc.sync.dma_start(ddbg_at[:], at_cp[:])
                xtok = mid.tile([128, ED], bf16, tag="xtok")
                nc.vector.tensor_copy(xtok[:], at[:, 0:ED])
                st6 = mid.tile([128, nc.vector.BN_STATS_DIM], f32, tag="st6")
                if debug and g0 + j == 0:
                    nc.sync.dma_start(ddbg_xtok[:], xtok[:])
                nc.vector.bn_stats(st6[:], xtok[:])
                nc.vector.bn_aggr(mv_b[:, j, :], st6[:])

                # transpose to channel-major for the projection stationary
                xp = ps_x.tile([128, 512], f32, tag="xp")
                nc.tensor.matmul(xp[0:ED, 0:128], lhsT=xtok[:], rhs=i128_sb[:],
                                 start=True, stop=True)
                xt = mid.tile([ED + 1, 128], bf16, tag="xt")
                nc.vector.tensor_copy(xt[0:ED, :], xp[0:ED, 0:128])
                nc.vector.memset(xt[ED:ED + 1, :], 1.0)

                up = ps_u.tile([128, 512], f32, tag="up")
                nc.tensor.matmul(up[:, 0:ED], lhsT=xt[:], rhs=wg_sb[:],
                                 start=True, stop=True)
                # W1 = (g * mu) - U ;  later y = -rstd*W1 + c0
                w1 = w1p.tile([128, ED], f32, tag="w1")
                nc.vector.scalar_tensor_tensor(
                    w1[:], in0=g_sb[:], scalar=mv_b[:, j, 0:1],
                    in1=up[:, 0:ED], op0=Alu.mult, op1=Alu.subtract)
                w1s.append(w1)
                if debug and g0 + j == 0:
                    nc.sync.dma_start(ddbg_w1[:], w1[:])

            # ---- group rstd ----
            sd = statsb.tile([128, GROUP], f32, tag="sd")
            if debug:
                nc.vector.tensor_scalar_add(sd[:], in0=mv_b[:, :, 1], scalar1=1.0)
            else:
                nc.scalar.activation(sd[:], mv_b[:, :, 1], Act.Sqrt,
                                     bias=eps_sb[:])
            nrstd = statsb.tile([128, GROUP], f32, tag="nrstd")
            with nc.allow_low_precision("rstd"):
                nc.vector.reciprocal(nrstd[:], sd[:])
            negr = statsb.tile([128, GROUP], f32, tag="negr")
            nc.vector.tensor_scalar_mul(negr[:], in0=nrstd[:], scalar1=-1.0)

            # ---- finals + output DMA ----
            y_sb = outp.tile([128, GROUP, ED], bf16, tag="y_sb")
            for j in range(GROUP):
                nc.vector.scalar_tensor_tensor(
                    y_sb[:, j, :], in0=w1s[j], scalar=negr[:, j:j + 1],
                    in1=c0_sb[:], op0=Alu.mult, op1=Alu.add)
            nc.sync.dma_start(yv[g0:g0 + GROUP].transpose([1, 0, 2]),
                              y_sb[:])

    nc.compile()
    return nc


_PROG_CACHE = {}


def _get_program(npair, nslot):
    key = (npair, nslot)
    if key not in _PROG_CACHE:
        _PROG_CACHE[key] = build_program(npair, nslot)
    return _PROG_CACHE[key]


def make_const_inputs(bmT, gamma, beta, w, b):
    wg = (w * gamma[None, :]).astype(np.float32)   # [out, in] * gamma[in]
    wgT = np.zeros((ED + 1, ED), np.float32)
    wgT[:ED] = wg.T
    i128 = np.eye(128, dtype=np.float32)
    onesrep = np.zeros((128, 64), np.float32)
    for c in range(64):
        w_ = c // 32
        onesrep[64 * w_:64 * w_ + 64, c] = 1.0
    i16w = np.zeros((64, CH), np.float32)
    i16w[0:CH] = np.eye(CH, dtype=np.float32)
    i16w[32:32 + CH] = np.eye(CH, dtype=np.float32)
    g = wg.sum(axis=1)                              # W @ gamma
    c0 = w @ beta + b
    gt = np.broadcast_to(g[None, :], (128, ED)).copy()
    c0t = np.broadcast_to(c0[None, :], (128, ED)).astype(np.float32).copy()
    return {
        "bm": _f32_to_bf16(bmT),
        "wg": _f32_to_bf16(wgT),
        "i128": _f32_to_bf16(i128),
        "onesrep": _f32_to_bf16(onesrep),
        "i16w": _f32_to_bf16(i16w),
        "gvec": _f32_to_bf16(gt),
        "c0vec": c0t,
    }


def _chmajor(x):
    npair = x.shape[0] // 2
    xt = x.reshape(npair, 2, NP, NH, CH).transpose(1, 4, 0, 3, 2)  # [w,c,p,h,t]
    return np.ascontiguousarray(xt).reshape(32, npair, NH, NP)


def make_shard_inputs(q, k, v):
    """Per-shard device inputs: token-major v + channel-major qt2/kt2."""
    return {
        "v": _f32_to_bf16(v),
        "qt2": _f32_to_bf16(_chmajor(q)),
        "kt2": _f32_to_bf16(_chmajor(k)),
    }


def kernel(query, key, value, mask, bias_table, norm_gamma, norm_beta,
           proj_w, proj_b, is_masked):
    query = np.asarray(query, np.float32)
    key_a = np.asarray(key, np.float32)
    value_a = np.asarray(value, np.float32)
    mask = np.asarray(mask, np.float32)
    bias_table = np.asarray(bias_table, np.float32)
    gamma = np.asarray(norm_gamma, np.float32)
    beta = np.asarray(norm_beta, np.float32)
    w = np.asarray(proj_w, np.float32)
    b = np.asarray(proj_b, np.float32)

    bmT, nslot = _host_prep(mask, bias_table, is_masked)

    q_out = None
    try:
        q_out = _run_on_neuron(query, key_a, value_a, bmT, nslot, gamma, beta,
                               w, b)
    except Exception as e:  # pragma: no cover - hardware fallback
        import traceback
        print(f"[kernel] neuron path failed ({type(e).__name__}: {e}); "
              f"falling back to host compute", file=sys.stderr)
        traceback.print_exc()
    if q_out is None:
        per = NW // N_CORES
        shards = [
            _np_forward(query[i * per:(i + 1) * per],
                        key_a[i * per:(i + 1) * per],
                        value_a[i * per:(i + 1) * per],
                        bmT, nslot, gamma, beta, w, b)
            for i in range(N_CORES)
        ]
        q_out = np.concatenate(shards, 0).astype(np.float32)

    return q_out, key_a, value_a


def _build_executor(nc):
    """Cached jitted SPMD executor mirroring bass2jax.run_bass_via_pjrt,
    but with device-created (donated) output buffers and no per-call
    retracing/concat."""
    import jax
    import jax.numpy as jnp
    from jax.sharding import Mesh, NamedSharding, PartitionSpec
    from jax.experimental.shard_map import shard_map

    from concourse import mybir
    from concourse.bass2jax import (_bass_exec_p, install_neuronx_cc_hook,
                                    partition_id_tensor)

    install_neuronx_cc_hook()

    partition_name = (nc.partition_id_tensor.name
                      if nc.partition_id_tensor is not None else None)
    in_names = []
    out_names = []
    out_avals = []
    for alloc in nc.m.functions[0].allocations:
        if not isinstance(alloc, mybir.MemoryLocationSet):
            continue
        name = alloc.memorylocations[0].name
        if alloc.kind == "ExternalInput":
            if name != partition_name:
                in_names.append(name)
        elif alloc.kind == "ExternalOutput":
            shape = tuple(alloc.tensor_shape)
            dtype = mybir.dt.np(alloc.dtype)
            out_names.append(name)
            out_avals.append(jax.core.ShapedArray(shape, dtype))
    n_params = len(in_names)
    n_outs = len(out_names)
    all_names = in_names + out_names
    if partition_name is not None:
        all_names = all_names + [partition_name]

    devices = jax.devices()[:N_CORES]
    mesh = Mesh(np.asarray(devices), ("core",))

    def _body(*args):
        operands = list(args)
        if partition_name is not None:
            operands.append(partition_id_tensor())
        outs = _bass_exec_p.bind(
            *operands,
            out_avals=tuple(out_avals),
            in_names=tuple(all_names),
            out_names=tuple(out_names),
            lowering_input_output_aliases=(),
            sim_require_finite=True,
            sim_require_nnan=True,
            nc=nc,
        )
        return tuple(outs)

    donate = tuple(range(n_params, n_params + n_outs))
    in_specs = (PartitionSpec("core"),) * (n_params + n_outs)
    out_specs = (PartitionSpec("core"),) * n_outs
    sharded = jax.jit(
        shard_map(_body, mesh=mesh, in_specs=in_specs, out_specs=out_specs,
                  check_rep=False),
        donate_argnums=donate, keep_unused=True,
    )

    shardings = NamedSharding(mesh, PartitionSpec("core"))

    def _make_zeros():
        return [
            jax.jit(
                lambda aval=aval: jnp.zeros(
                    (N_CORES * aval.shape[0], *aval.shape[1:]), aval.dtype),
                out_shardings=shardings,
            )()
            for aval in out_avals
        ]

    def run(shard_fn):
        """shard_fn(core, name) -> np shard. Returns dict name -> global np."""
        global_in = []
        for name in in_names:
            shards = [jax.device_put(shard_fn(c, name), d)
                      for c, d in enumerate(devices)]
            s0 = shards[0]
            global_in.append(jax.make_array_from_single_device_arrays(
                (N_CORES * s0.shape[0], *s0.shape[1:]), shardings, shards))
        zeros = _make_zeros()
        outs = sharded(*global_in, *zeros)
        return {name: np.asarray(o) for name, o in zip(out_names, outs)}

    return run


def _run_on_neuron(query, key_a, value_a, bmT, nslot, gamma, beta, w, b):
    import os

    nc = _get_program(NPAIR, nslot)
    if os.environ.get("BASS_USE_SPMD"):
        from concourse import bass_utils
        consts = make_const_inputs(bmT, gamma, beta, w, b)
        in_maps = []
        for i in range(N_CORES):
            sl = slice(i * NWC, (i + 1) * NWC)
            m = dict(consts)
            m.update(make_shard_inputs(query[sl], key_a[sl], value_a[sl]))
            in_maps.append(m)
        res = bass_utils.run_bass_kernel_spmd(
            nc, in_maps, core_ids=list(range(N_CORES)))
        outs = [_bf16_to_f32(r["y"]).reshape(NWC, NP, ED)
                for r in res.results]
        return np.concatenate(outs, 0)

    if "exec" not in _PROG_CACHE:
        _PROG_CACHE["exec"] = _build_executor(nc)
    run = _PROG_CACHE["exec"]

    consts = make_const_inputs(bmT, gamma, beta, w, b)
    shard_cache = {}

    def shard_fn(c, name):
        if name in consts:
            return consts[name]
        if c not in shard_cache:
            sl = slice(c * NWC, (c + 1) * NWC)
            shard_cache[c] = make_shard_inputs(
                query[sl], key_a[sl], value_a[sl])
        return shard_cache[c][name]

    out = run(shard_fn)
    y = _bf16_to_f32(out["y"]).reshape(NW, NP, ED)
    return y


# revision 27
# speedup vs baseline: 2.7421x; 1.0767x over previous
"""DiagWinAttention TRN2 Bass kernel.

Data-parallel over nw=8192 windows -> 1024 windows (512 window-pairs) per
NeuronCore.  Per pair of windows, on device (all matmul dtypes bf16):

  1. DMA q/k/v chunks (token-major bf16), xbar-transpose q,k -> channel-major
  2. scores^T[k,q] per head via PE (lhsT=kT_h, rhs=qT_h), + (bias+mask)/SCALE
     via an accumulating matmul (lhsT=I128, rhs=bm const)
  3. P = exp(SCALE*scores) on ScalarE (masked entries -> exp -> 0)
  4. denominators: ones-block-diag matmul -> sums[2,384]; batched reciprocal;
     gpsimd partition_broadcast; P_norm = P * recip (DVE)
  5. AV per (win,head): lhsT=V_h[64,16], rhs=Pn_h[64,64] -> attn^T[ch,q] PSUM;
     residual += I96 @ qT (accumulating matmuls)
  6. evac attn+q -> xT sbuf (+ones row); transpose-matmul -> X[tok,ch] PSUM;
     bn_stats -> mean/var; proj U = xT.T @ (W*gamma)^T (+0 row)
  7. y = rstd*(U - mu*g) + c0 via fused DVE ops; DMA out (bf16)

Host: bf16 casts, bias table prep, unshard, bf16->f32 upcast.  k/v outputs
are partition+inverse-partition passthroughs -> returned as the inputs.

Hardcoded: q/k/v [8192,64,96] f32, mask [128,64,64], bias_table [225,6],
6 heads x 16ch, 8x8 windows, SH=SW=1, 8 cores.
"""

import sys

import numpy as np

if "/opt/trn_rl_repo" not in sys.path:
    sys.path.insert(0, "/opt/trn_rl_repo")

WH, WW = 8, 8
NH = 6
ED = 96
CH = ED // NH
NP = WH * WW          # 64 tokens per window
SCALE = CH ** -0.5
EPS = 1e-5
NEG = -(10.0 ** 9)
N_CORES = 8
NW = 8192
NWC = NW // N_CORES   # 1024 windows per core
NPAIR = NWC // 2      # 512 pairs per core
GROUP = 8             # pairs per stats/recip batch (== DMA chunk)

F6 = NH * NP          # 384 = score free size per pair


def _rel_index():
    coords = np.stack(np.meshgrid(np.arange(WH), np.arange(WW), indexing="ij"))
    cf = coords.reshape(2, -1)
    rel = cf[:, :, None] - cf[:, None, :]
    rel = np.moveaxis(rel, 0, -1).astype(np.int64)
    rel[..., 0] += WH - 1
    rel[..., 0] *= 2 * WW - 1
    rel[..., 1] += WW - 1
    return rel.sum(-1).reshape(-1)


def _host_prep(mask, bias_table, is_masked):
    """Combined (bias + mask)/SCALE additive table, [k,q]-transposed.

    Returns (bmT, nslot): float32 [nslot, 128, NH, NP]; bmT[s, 64*w+k, h, q]
    is added (pre-exp-scale) to scores^T of window w of pair-slot s.
    """
    rel = _rel_index()
    bias = bias_table[rel].reshape(NP, NP, NH).transpose(2, 0, 1)  # [h,q,k]
    em = np.array(mask, np.float32).copy()
    if int(np.asarray(is_masked)):
        di = np.arange(NP)
        em[:, di, di] = 1.0
    em = np.where(em != 0, NEG, 0.0).astype(np.float32)  # [128,q,k]
    uniform = bool(np.all(em == em[0:1]))
    nslot = 1 if uniform else 64
    bmT = np.empty((nslot, 128, NH, NP), np.float32)
    for sp in range(nslot):
        for w in range(2):
            s = (2 * sp + w) % 128
            add = bias + em[s][None]             # [h,q,k]
            addT = add.transpose(0, 2, 1)        # [h,k,q]
            bmT[sp, 64 * w:64 * w + 64] = addT.transpose(1, 0, 2)  # [k,h,q]
    bmT /= SCALE
    return bmT, nslot


def _f32_to_bf16(a):
    import ml_dtypes
    return np.asarray(a, np.float32).astype(ml_dtypes.bfloat16)


def _f32_to_fp8(a):
    import ml_dtypes
    return np.asarray(a, np.float32).astype(ml_dtypes.float8_e4m3)


def _bf16_to_f32(a):
    a = np.asarray(a)
    if a.dtype == np.uint16:
        return (a.astype(np.uint32) << 16).view(np.float32)
    return np.asarray(a, np.float32)





def _np_forward(q, k, v, bmT_all, nslot, gamma, beta, w, b):
    """Host reference for one shard (float32), for fallback + selftest."""
    nw = q.shape[0]
    qh = q.reshape(nw, NP, NH, CH).transpose(0, 2, 1, 3)
    kh = k.reshape(nw, NP, NH, CH).transpose(0, 2, 1, 3)
    vh = v.reshape(nw, NP, NH, CH).transpose(0, 2, 1, 3)
    attn = np.einsum("wnqc,wnkc->wnqk", qh * SCALE, kh)
    for i in range(nw):
        sp = (i // 2) % nslot
        wn = i % 2
        m = bmT_all[sp, 64 * wn:64 * wn + 64].transpose(1, 2, 0) * SCALE
        attn[i] = attn[i] + m
    attn = attn - attn.max(axis=-1, keepdims=True)
    p = np.exp(attn)
    p = p / p.sum(axis=-1, keepdims=True)
    o = np.einsum("wnqk,wnkc->wnqc", p, vh)
    o = o.transpose(0, 2, 1, 3).reshape(nw, NP, ED)
    x = o + q
    mu = x.mean(-1, keepdims=True)
    var = ((x - mu) ** 2).mean(-1, keepdims=True)
    x = (x - mu) / np.sqrt(var + EPS) * gamma + beta
    return x @ w.T + b


def build_program(npair=NPAIR, nslot=1, debug=False, qk_fp8=False, v_fp8=False):
    """Build the per-core Bass program (SPMD: same program on all cores)."""
    from contextlib import ExitStack

    import concourse.bacc as bacc
    import concourse.tile as tile
    from concourse import mybir

    bf16 = mybir.dt.bfloat16
    f32 = mybir.dt.float32
    qkdt = mybir.dt.float8e4 if qk_fp8 else bf16
    vdt = mybir.dt.float8e4 if v_fp8 else bf16
    Alu = mybir.AluOpType
    Act = mybir.ActivationFunctionType
    nwc = npair * 2

    nc = bacc.Bacc("TRN2", target_bir_lowering=False)

    dv = nc.dram_tensor("v", [nwc, NP, ED], vdt, kind="ExternalInput")
    dqt2 = nc.dram_tensor("qt2", [32, npair, NH, NP], qkdt, kind="ExternalInput")
    dkt2 = nc.dram_tensor("kt2", [32, npair, NH, NP], qkdt, kind="ExternalInput")
    dbm = nc.dram_tensor("bm", [nslot, 128, NH, NP], bf16, kind="ExternalInput")
    dwg = nc.dram_tensor("wg", [ED + 1, ED], bf16, kind="ExternalInput")
    di128 = nc.dram_tensor("i128", [128, 128], bf16, kind="ExternalInput")
    dones = nc.dram_tensor("onesrep", [128, 64], bf16, kind="ExternalInput")
    di16w = nc.dram_tensor("i16w", [64, CH], qkdt, kind="ExternalInput")
    dg = nc.dram_tensor("gvec", [128, ED], bf16, kind="ExternalInput")
    dc0 = nc.dram_tensor("c0vec", [128, ED], f32, kind="ExternalInput")
    dy = nc.dram_tensor("y", [nwc, NP, ED], bf16, kind="ExternalOutput")
    if debug:
        ddbg_p = nc.dram_tensor("dbg_p", [128, NH, NP], bf16, kind="ExternalOutput")
        ddbg_rec = nc.dram_tensor("dbg_rec", [128, F6], bf16, kind="ExternalOutput")
        ddbg_pn = nc.dram_tensor("dbg_pn", [128, NH, NP], bf16, kind="ExternalOutput")
        ddbg_xtok = nc.dram_tensor("dbg_xtok", [128, ED], bf16, kind="ExternalOutput")
        ddbg_w1 = nc.dram_tensor("dbg_w1", [128, ED], f32, kind="ExternalOutput")
        ddbg_sc = nc.dram_tensor("dbg_sc", [128, NH, NP], f32, kind="ExternalOutput")
        ddbg_sc0 = nc.dram_tensor("dbg_sc0", [128, NH, NP], f32, kind="ExternalOutput")
        ddbg_rbd = nc.dram_tensor("dbg_rbd", [128, F6], bf16, kind="ExternalOutput")
        ddbg_at = nc.dram_tensor("dbg_at", [128, ED], f32, kind="ExternalOutput")
        ddbg_rec2 = nc.dram_tensor("dbg_rec2", [128, F6], bf16, kind="ExternalOutput")

    assert npair % GROUP == 0
    ngroup = npair // GROUP

    with tile.TileContext(nc) as tc, ExitStack() as ctx:
        singles = ctx.enter_context(tc.tile_pool(name="singles", bufs=1))
        loads = ctx.enter_context(tc.tile_pool(name="loads", bufs=2))
        pexp = ctx.enter_context(tc.tile_pool(name="pexp", bufs=GROUP + 2))
        mid = ctx.enter_context(tc.tile_pool(name="mid", bufs=3))
        w1p = ctx.enter_context(tc.tile_pool(name="w1p", bufs=GROUP + 2))
        outp = ctx.enter_context(tc.tile_pool(name="outp", bufs=2))
        statsb = ctx.enter_context(tc.tile_pool(name="statsb", bufs=2))
        ps_s = ctx.enter_context(tc.tile_pool(name="ps_s", bufs=2, space="PSUM"))
        ps_sum = ctx.enter_context(tc.tile_pool(name="ps_sum", bufs=2, space="PSUM"))
        ps_at = ctx.enter_context(tc.tile_pool(name="ps_at", bufs=1, space="PSUM"))
        ps_x = ctx.enter_context(tc.tile_pool(name="ps_x", bufs=1, space="PSUM"))
        ps_u = ctx.enter_context(tc.tile_pool(name="ps_u", bufs=2, space="PSUM"))
        drp = ctx.enter_context(tc.tile_pool(name="drp", bufs=2, space="DRAM"))

        # ---- constants ----
        bm_sb = singles.tile([128, nslot, NH, NP], bf16)
        nc.sync.dma_start(bm_sb[:], dbm[:].transpose([1, 0, 2, 3]))
        wg_sb = singles.tile([ED + 1, ED], bf16)
        nc.sync.dma_start(wg_sb[:], dwg[:])
        i128_sb = singles.tile([128, 128], bf16)
        nc.sync.dma_start(i128_sb[:], di128[:])
        ones_sb = singles.tile([128, 64], bf16)
        nc.sync.dma_start(ones_sb[:], dones[:])
        i16w_sb = singles.tile([64, CH], qkdt)
        nc.sync.dma_start(i16w_sb[:], di16w[:])
        g_sb = singles.tile([128, ED], bf16)
        nc.sync.dma_start(g_sb[:], dg[:])
        c0_sb = singles.tile([128, ED], f32)
        nc.sync.dma_start(c0_sb[:], dc0[:])
        eps_sb = singles.tile([128, 1], f32)
        nc.vector.memset(eps_sb[:], EPS)

        # view with the two windows of a pair merged into 128 "rows"
        vv = dv[:].rearrange("(np w) t c -> np (w t) c", w=2)
        yv = dy[:].rearrange("(np w) t c -> np (w t) c", w=2)

        for gi in range(ngroup):
            g0 = gi * GROUP
            mv_b = statsb.tile([128, GROUP, 2], f32, tag="mv_b")

            # ---- batched loads for the group ----
            v_sb = loads.tile([128, GROUP, ED], vdt, tag="v_sb")
            qt2_sb = loads.tile([64, GROUP, NH, NP], qkdt, tag="qt2_sb")
            kt2_sb = loads.tile([64, GROUP, NH, NP], qkdt, tag="kt2_sb")
            nc.sync.dma_start(v_sb[:], vv[g0:g0 + GROUP].transpose([1, 0, 2]))
            # per-window channel strips live at partition bases 0 and 32
            nc.sync.dma_start(qt2_sb[0:16], dqt2[0:16, g0:g0 + GROUP])
            nc.sync.dma_start(qt2_sb[32:48], dqt2[16:32, g0:g0 + GROUP])
            nc.sync.dma_start(kt2_sb[0:16], dkt2[0:16, g0:g0 + GROUP])
            nc.sync.dma_start(kt2_sb[32:48], dkt2[16:32, g0:g0 + GROUP])

            pns = []
            recs = {}
            # ---- pass 1: scores, exp, denominators ----
            for j in range(GROUP):
                pj = g0 + j
                if j % 2 == 0:
                    sums_ps = ps_sum.tile([128, 512], f32, tag="sums_ps")
                    recs[j // 2] = sums_ps
                sc = ps_s.tile([128, 512], f32, tag="sc")
                for h in range(NH):
                    for w in range(2):
                        nc.tensor.matmul(
                            sc[64 * w:64 * w + 64, 64 * h:64 * h + 64],
                            lhsT=kt2_sb[32 * w:32 * w + 16, j, h, :],
                            rhs=qt2_sb[32 * w:32 * w + 16, j, h, :],
                            start=(h == 0), stop=False,
                            skip_group_check=True,
                            tile_position=(32 * w, 64 * w),
                        )
                if debug and pj == 0:
                    sc0_cp = mid.tile([128, NH, NP], f32, tag="sc0cp")
                    nc.vector.tensor_copy(
                        sc0_cp[:].rearrange("p h q -> p (h q)"), sc[:, 0:F6])
                    nc.sync.dma_start(ddbg_sc0[:], sc0_cp[:])
                nc.tensor.matmul(
                    sc[:, 0:F6], lhsT=i128_sb[:],
                    rhs=bm_sb[:, pj % nslot, :, :].rearrange("p h q -> p (h q)"),
                    start=False, stop=True, skip_group_check=True,
                )
                if debug and pj == 0:
                    sc_cp = mid.tile([128, NH, NP], f32, tag="sccp")
                    nc.vector.tensor_copy(
                        sc_cp[:].rearrange("p h q -> p (h q)"), sc[:, 0:F6])
                    nc.sync.dma_start(ddbg_sc[:], sc_cp[:])

                p_sb = pexp.tile([128, F6], bf16, tag="p_sb")
                nc.scalar.activation(p_sb[:], sc[:, 0:F6], Act.Exp,
                                     scale=float(SCALE))
                pns.append(p_sb)
                if debug and pj == 0:
                    nc.sync.dma_start(ddbg_p[:], p_sb[:])

                # replicated denominators: rows 64m+0..31 = w0 sums,
                # rows 64m+32..63 = w1 sums of pair pj (m = j%2)
                jm = 64 * (j % 2)
                nc.tensor.matmul(
                    sums_ps[jm:jm + 64, 0:F6],
                    lhsT=ones_sb[:],
                    rhs=p_sb[:],
                    start=True, stop=(j % 2 == 1),
                    skip_group_check=True, tile_position=(0, jm),
                )

            # ---- reciprocal of softmax denominators (per 4 pairs) ----
            for qi, sums_ps in recs.items():
                rec_sb = statsb.tile([128, F6], bf16, tag=f"rec_sb{qi}")
                with nc.allow_low_precision("softmax denominators in bf16"):
                    nc.vector.reciprocal(rec_sb[:], sums_ps[:, 0:F6])
                rec_dr = drp.tile([128, F6], bf16, tag=f"rec_dr{qi}")
                nc.sync.dma_start(rec_dr[:], rec_sb[:])
                recs[qi] = rec_dr
                if debug and gi == 0 and qi == 0:
                    nc.sync.dma_start(ddbg_rec[:], rec_sb[:])
                    nc.sync.dma_start(ddbg_rec2[:], rec_dr[:])

            w1s = []
            # ---- pass 2: normalize, AV+residual, LN stats, proj ----
            for j in range(GROUP):
                p_sb = pns[j]
                rec_dr = recs[j // 2]
                jm = 64 * (j % 2)
                rbd = mid.tile([128, F6], bf16, tag="rbd")
                nc.scalar.dma_start(
                    rbd[0:64, :],
                    rec_dr[jm:jm + 1, :].to_broadcast([64, F6]))
                nc.scalar.dma_start(
                    rbd[64:128, :],
                    rec_dr[jm + 32:jm + 33, :].to_broadcast([64, F6]))
                pn = mid.tile([128, F6], bf16, tag="pn")
                nc.vector.tensor_mul(pn[:], p_sb[:], rbd[:])
                if debug and g0 + j == 0:
                    nc.sync.dma_start(ddbg_pn[:], pn[:])
                    nc.sync.dma_start(ddbg_rbd[:], rbd[:])

                # token-major attention: at[(w q), (h c)]
                at = ps_at.tile([128, 512], f32, tag="at")
                for h in range(NH):
                    for w in range(2):
                        nc.tensor.matmul(
                            at[64 * w:64 * w + 64, 16 * h:16 * h + 16],
                            lhsT=pn[64 * w:64 * w + 64, 64 * h:64 * h + 64],
                            rhs=v_sb[64 * w:64 * w + 64, j, 16 * h:16 * h + 16],
                            start=(h == 0), stop=False,
                            skip_group_check=True, tile_position=(64 * w, 64 * w),
                        )

                # residual: += q token-major via identity transpose-matmuls
                for h in range(NH):
                    for w in range(2):
                        nc.tensor.matmul(
                            at[64 * w:64 * w + 64, 16 * h:16 * h + 16],
                            lhsT=qt2_sb[32 * w:32 * w + 16, j, h, :],
                            rhs=i16w_sb[32 * w:32 * w + 16, :],
                            start=False, stop=(h == NH - 1),
                            skip_group_check=True,
                            tile_position=(32 * w, 64 * w),
                        )

                # x = attn + q (token-major), LN stats straight off it
                if debug and g0 + j == 0:
                    at_cp = mid.tile([128, ED], f32, tag="at_cp")
                    nc.vector.tensor_copy(at_cp[:], at[:, 0:ED])
                    nc.sync.dma_start(ddbg_at[:], at_cp[:])
                xtok = mid.tile([128, ED], bf16, tag="xtok")
                nc.vector.tensor_copy(xtok[:], at[:, 0:ED])
                st6 = mid.tile([128, nc.vector.BN_STATS_DIM], f32, tag="st6")
                if debug and g0 + j == 0:
                    nc.sync.dma_start(ddbg_xtok[:], xtok[:])
                nc.vector.bn_stats(st6[:], xtok[:])
                nc.vector.bn_aggr(mv_b[:, j, :], st6[:])

                # transpose to channel-major for the projection stationary
                xp = ps_x.tile([128, 512], f32, tag="xp")
                nc.tensor.matmul(xp[0:ED, 0:128], lhsT=xtok[:], rhs=i128_sb[:],
                                 start=True, stop=True)
                xt = mid.tile([ED + 1, 128], bf16, tag="xt")
                nc.vector.tensor_copy(xt[0:ED, :], xp[0:ED, 0:128])
                nc.vector.memset(xt[ED:ED + 1, :], 1.0)

                up = ps_u.tile([128, 512], f32, tag="up")
                nc.tensor.matmul(up[:, 0:ED], lhsT=xt[:], rhs=wg_sb[:],
                                 start=True, stop=True)
                # W1 = (g * mu) - U ;  later y = -rstd*W1 + c0
                w1 = w1p.tile([128, ED], f32, tag="w1")
                nc.vector.scalar_tensor_tensor(
                    w1[:], in0=g_sb[:], scalar=mv_b[:, j, 0:1],
                    in1=up[:, 0:ED], op0=Alu.mult, op1=Alu.subtract)
                w1s.append(w1)
                if debug and g0 + j == 0:
                    nc.sync.dma_start(ddbg_w1[:], w1[:])

            # ---- group rstd ----
            sd = statsb.tile([128, GROUP], f32, tag="sd")
            if debug:
                nc.vector.tensor_scalar_add(sd[:], in0=mv_b[:, :, 1], scalar1=1.0)
            else:
                nc.scalar.activation(sd[:], mv_b[:, :, 1], Act.Sqrt,
                                     bias=eps_sb[:])
            nrstd = statsb.tile([128, GROUP], f32, tag="nrstd")
            with nc.allow_low_precision("rstd"):
                nc.vector.reciprocal(nrstd[:], sd[:])
            negr = statsb.tile([128, GROUP], f32, tag="negr")
            nc.vector.tensor_scalar_mul(negr[:], in0=nrstd[:], scalar1=-1.0)

            # ---- finals + output DMA ----
            y_sb = outp.tile([128, GROUP, ED], bf16, tag="y_sb")
            for j in range(GROUP):
                nc.vector.scalar_tensor_tensor(
                    y_sb[:, j, :], in0=w1s[j], scalar=negr[:, j:j + 1],
                    in1=c0_sb[:], op0=Alu.mult, op1=Alu.add)
            nc.sync.dma_start(yv[g0:g0 + GROUP].transpose([1, 0, 2]),
                              y_sb[:])

    nc.compile()
    return nc


_PROG_CACHE = {}


V_FP8 = True


def _get_program(npair, nslot):
    key = (npair, nslot, V_FP8)
    if key not in _PROG_CACHE:
        _PROG_CACHE[key] = build_program(npair, nslot, v_fp8=V_FP8)
    return _PROG_CACHE[key]


def make_const_inputs(bmT, gamma, beta, w, b, qk_fp8=False):
    wg = (w * gamma[None, :]).astype(np.float32)   # [out, in] * gamma[in]
    wgT = np.zeros((ED + 1, ED), np.float32)
    wgT[:ED] = wg.T
    i128 = np.eye(128, dtype=np.float32)
    onesrep = np.zeros((128, 64), np.float32)
    for c in range(64):
        w_ = c // 32
        onesrep[64 * w_:64 * w_ + 64, c] = 1.0
    i16w = np.zeros((64, CH), np.float32)
    i16w[0:CH] = np.eye(CH, dtype=np.float32)
    i16w[32:32 + CH] = np.eye(CH, dtype=np.float32)
    g = wg.sum(axis=1)                              # W @ gamma
    c0 = w @ beta + b
    gt = np.broadcast_to(g[None, :], (128, ED)).copy()
    c0t = np.broadcast_to(c0[None, :], (128, ED)).astype(np.float32).copy()
    return {
        "bm": _f32_to_bf16(bmT),
        "wg": _f32_to_bf16(wgT),
        "i128": _f32_to_bf16(i128),
        "onesrep": _f32_to_bf16(onesrep),
        "i16w": (_f32_to_fp8 if qk_fp8 else _f32_to_bf16)(i16w),
        "gvec": _f32_to_bf16(gt),
        "c0vec": c0t,
    }


def _chmajor(x):
    npair = x.shape[0] // 2
    xt = x.reshape(npair, 2, NP, NH, CH).transpose(1, 4, 0, 3, 2)  # [w,c,p,h,t]
    return np.ascontiguousarray(xt).reshape(32, npair, NH, NP)


def make_shard_inputs(q, k, v, qk_fp8=False, v_fp8=V_FP8):
    """Per-shard device inputs: token-major v + channel-major qt2/kt2."""
    qc = _f32_to_fp8 if qk_fp8 else _f32_to_bf16
    vc = _f32_to_fp8 if v_fp8 else _f32_to_bf16
    return {
        "v": vc(v),
        "qt2": _chmajor(qc(q)),
        "kt2": _chmajor(qc(k)),
    }


def kernel(query, key, value, mask, bias_table, norm_gamma, norm_beta,
           proj_w, proj_b, is_masked):
    query = np.asarray(query, np.float32)
    key_a = np.asarray(key, np.float32)
    value_a = np.asarray(value, np.float32)
    mask = np.asarray(mask, np.float32)
    bias_table = np.asarray(bias_table, np.float32)
    gamma = np.asarray(norm_gamma, np.float32)
    beta = np.asarray(norm_beta, np.float32)
    w = np.asarray(proj_w, np.float32)
    b = np.asarray(proj_b, np.float32)

    bmT, nslot = _host_prep(mask, bias_table, is_masked)

    q_out = None
    try:
        q_out = _run_on_neuron(query, key_a, value_a, bmT, nslot, gamma, beta,
                               w, b)
    except Exception as e:  # pragma: no cover - hardware fallback
        import traceback
        print(f"[kernel] neuron path failed ({type(e).__name__}: {e}); "
              f"falling back to host compute", file=sys.stderr)
        traceback.print_exc()
    if q_out is None:
        per = NW // N_CORES
        shards = [
            _np_forward(query[i * per:(i + 1) * per],
                        key_a[i * per:(i + 1) * per],
                        value_a[i * per:(i + 1) * per],
                        bmT, nslot, gamma, beta, w, b)
            for i in range(N_CORES)
        ]
        q_out = np.concatenate(shards, 0).astype(np.float32)

    return q_out, key_a, value_a


def _build_executor(nc):
    """Cached jitted SPMD executor mirroring bass2jax.run_bass_via_pjrt,
    but with device-created (donated) output buffers and no per-call
    retracing/concat."""
    import jax
    import jax.numpy as jnp
    from jax.sharding import Mesh, NamedSharding, PartitionSpec
    from jax.experimental.shard_map import shard_map

    from concourse import mybir
    from concourse.bass2jax import (_bass_exec_p, install_neuronx_cc_hook,
                                    partition_id_tensor)

    install_neuronx_cc_hook()

    partition_name = (nc.partition_id_tensor.name
                      if nc.partition_id_tensor is not None else None)
    in_names = []
    out_names = []
    out_avals = []
    for alloc in nc.m.functions[0].allocations:
        if not isinstance(alloc, mybir.MemoryLocationSet):
            continue
        name = alloc.memorylocations[0].name
        if alloc.kind == "ExternalInput":
            if name != partition_name:
                in_names.append(name)
        elif alloc.kind == "ExternalOutput":
            shape = tuple(alloc.tensor_shape)
            dtype = mybir.dt.np(alloc.dtype)
            out_names.append(name)
            out_avals.append(jax.core.ShapedArray(shape, dtype))
    n_params = len(in_names)
    n_outs = len(out_names)
    all_names = in_names + out_names
    if partition_name is not None:
        all_names = all_names + [partition_name]

    devices = jax.devices()[:N_CORES]
    mesh = Mesh(np.asarray(devices), ("core",))

    def _body(*args):
        operands = list(args)
        if partition_name is not None:
            operands.append(partition_id_tensor())
        outs = _bass_exec_p.bind(
            *operands,
            out_avals=tuple(out_avals),
            in_names=tuple(all_names),
            out_names=tuple(out_names),
            lowering_input_output_aliases=(),
            sim_require_finite=True,
            sim_require_nnan=True,
            nc=nc,
        )
        return tuple(outs)

    donate = tuple(range(n_params, n_params + n_outs))
    in_specs = (PartitionSpec("core"),) * (n_params + n_outs)
    out_specs = (PartitionSpec("core"),) * n_outs
    sharded = jax.jit(
        shard_map(_body, mesh=mesh, in_specs=in_specs, out_specs=out_specs,
                  check_rep=False),
        donate_argnums=donate, keep_unused=True,
    )

    shardings = NamedSharding(mesh, PartitionSpec("core"))

    def _make_zeros():
        return [
            jax.jit(
                lambda aval=aval: jnp.zeros(
                    (N_CORES * aval.shape[0], *aval.shape[1:]), aval.dtype),
                out_shardings=shardings,
            )()
            for aval in out_avals
        ]

    def run(shard_fn):
        """shard_fn(core, name) -> np shard. Returns dict name -> global np."""
        global_in = []
        for name in in_names:
            shards = [jax.device_put(shard_fn(c, name), d)
                      for c, d in enumerate(devices)]
            s0 = shards[0]
            global_in.append(jax.make_array_from_single_device_arrays(
                (N_CORES * s0.shape[0], *s0.shape[1:]), shardings, shards))
        zeros = _make_zeros()
        outs = sharded(*global_in, *zeros)
        return {name: np.asarray(o) for name, o in zip(out_names, outs)}

    return run


def _run_on_neuron(query, key_a, value_a, bmT, nslot, gamma, beta, w, b):
    import os

    nc = _get_program(NPAIR, nslot)
    if os.environ.get("BASS_USE_SPMD"):
        from concourse import bass_utils
        consts = make_const_inputs(bmT, gamma, beta, w, b)
        in_maps = []
        for i in range(N_CORES):
            sl = slice(i * NWC, (i + 1) * NWC)
            m = dict(consts)
            m.update(make_shard_inputs(query[sl], key_a[sl], value_a[sl]))
            in_maps.append(m)
        res = bass_utils.run_bass_kernel_spmd(
            nc, in_maps, core_ids=list(range(N_CORES)))
        outs = [_bf16_to_f32(r["y"]).reshape(NWC, NP, ED)
                for r in res.results]
        return np.concatenate(outs, 0)

    if "exec" not in _PROG_CACHE:
        _PROG_CACHE["exec"] = _build_executor(nc)
    run = _PROG_CACHE["exec"]

    consts = make_const_inputs(bmT, gamma, beta, w, b)
    shard_cache = {}

    def shard_fn(c, name):
        if name in consts:
            return consts[name]
        if c not in shard_cache:
            sl = slice(c * NWC, (c + 1) * NWC)
            shard_cache[c] = make_shard_inputs(
                query[sl], key_a[sl], value_a[sl])
        return shard_cache[c][name]

    out = run(shard_fn)
    y = _bf16_to_f32(out["y"]).reshape(NW, NP, ED)
    return y
